# revision 15
# baseline (speedup 1.0000x reference)
"""Karras optimal denoiser on 8 Trainium2 NeuronCores — sparse-GEMM2 version.

Math: out_b = sum_i w_bi y_i / sum_i w_bi,  w_bi = exp((x_b.y_i - 0.5||y_i||^2)/sigma_b^2).
Softmax over N=50000 is extremely peaked (retrieval-knn): per core only ~700 of
6250 columns have weight mass >= tau=exp(-14) relative to the local row max.

Per-core plan (train_data sharded over N, flash-style host combine):
  Phase 1 (GEMM1, transposed): LamT[i, b] = (x_b*is2_b . y_i) + (-0.5||y_i||^2)*is2_b
    computed 128-column-group at a time: stationary = y^T k-tiles, moving = x~^T.
    The y2*is2 term enters as a K=3 matmul (hi/lo fp16 split of both factors).
    Per group: PSUM -> hi16 (fp16) + lo16 (fp16 residual, exact to ~2^-12),
    both DMA'd into spare columns of the ynat DRAM tensor (rows = train points);
    hi16 also kept in SBUF. Row maxes via 2 PE transposes + DVE reduce_max.
  Selection: keepscore_i = max_b (hi - gm~_b); columns with keepscore >= ln(tau)
    are compacted to a rank list by gpsimd sparse_gather ([16,F] wrapped layout;
    DRAM-bounce rearranges to/from it). Slots are rank-ordered => perfectly
    partition-balanced. Slot validity from num_found (HW tail is garbage).
  Gather: per slot-block j, one indirect DMA fetches 128 rows of
    ynat = [y row fp16 (3072) | hi (256) | lo (256)] -> 7KB rows at ~line rate.
  W'^T_j = exp(hi + lo - gm~) (invalid slots killed via -1e30 bias); den = ones^T W'.
  GEMM2: num = W'^T.T @ yhat over CMAX j-blocks, m-blocked (6 PSUM banks),
    pipelined against the gathers.
Outputs per core: num [2,128,3072], den [1,256], mx [128,2] (=gm~, scaled-logit
frame); host does the flash combine in that frame.
"""

import numpy as np
from contextlib import ExitStack

import concourse.bass as bass
import concourse.tile as tile
import concourse.mybir as mybir
from concourse import bacc
from concourse.bass_utils import run_bass_kernel_spmd
from concourse.masks import make_identity

dt = mybir.dt
Alu = mybir.AluOpType
Act = mybir.ActivationFunctionType

B, C, H, W_IMG = 256, 3, 32, 32
D = C * H * W_IMG            # 3072
N_TOTAL = 50000
N_CORES = 8
NS = N_TOTAL // N_CORES      # 6250 per core
NSP = 6272                   # padded to 49*128
KT = D // 128                # 24 contraction k-tiles
KH = KT // 2                 # 12 per stream half
G = NSP // 128               # 49 column groups
M_TILES = 2                  # 256 query rows = 2 partition tiles
CMAX = 8                     # survivor slot blocks (cap = 128*CMAX = 1024)
LNTAU = -14.0                # keep threshold on log-weights
DAUG = D + 512               # ynat row: y | hi | lo
PAD_Y2 = -15000.0            # pad-row -0.5||y||^2 sentinel (never selected; *4 stays in fp16)


def chunk_list(nsp=NSP):
    """128-multiple chunks, ramped so the first y DMAs land fast."""
    sizes = [128, 256, 384]
    out, off = [], 0
    for s in sizes:
        out.append((off, s))
        off += s
    while off < nsp:
        out.append((off, min(512, nsp - off)))
        off += 512
    return out


def build_nc():
    chunks = chunk_list()
    nc = bacc.Bacc("TRN2", target_bir_lowering=False, debug=False)

    # --- DRAM I/O ---
    y_d = [nc.dram_tensor(f"y_c{ci}", (2, 128, KH, csz), dt.float16, kind="ExternalInput").ap()
           for ci, (_, csz) in enumerate(chunks)]
    y2t_d = nc.dram_tensor("y2t", (3, NSP), dt.float16, kind="ExternalInput").ap()
    is2r_d = nc.dram_tensor("is2r", (3, 256), dt.float16, kind="ExternalInput").ap()
    xst_d = nc.dram_tensor("xst", (KT, 128, 256), dt.float16, kind="ExternalInput").ap()
    ynat_d = nc.dram_tensor("ynat", (NSP, DAUG), dt.float16, kind="ExternalInput").ap()

    num_d = nc.dram_tensor("num", (M_TILES, 128, D), dt.float32, kind="ExternalOutput").ap()
    den_d = nc.dram_tensor("den", (1, 256), dt.float32, kind="ExternalOutput").ap()
    mx_d = nc.dram_tensor("mx", (1, M_TILES, 128), dt.float32, kind="ExternalOutput").ap()
    scr1_d = nc.dram_tensor("scr1", (G, 128), dt.float32, kind="ExternalOutput").ap()
    scr2_d = nc.dram_tensor("scr2", (CMAX * 8, 16), dt.float32, kind="ExternalOutput").ap()

    with tile.TileContext(nc) as tc:
        with ExitStack() as ctx:
            small = ctx.enter_context(tc.tile_pool(name="small", bufs=1))
            xpool = ctx.enter_context(tc.tile_pool(name="x", bufs=1))
            ypool = ctx.enter_context(tc.tile_pool(name="ystream", bufs=4))
            hipool = ctx.enter_context(tc.tile_pool(name="hi", bufs=1))
            lopool = ctx.enter_context(tc.tile_pool(name="lo", bufs=3))
            spool = ctx.enter_context(tc.tile_pool(name="ssel", bufs=4))
            tpool = ctx.enter_context(tc.tile_pool(name="texp", bufs=3))
            wtp = ctx.enter_context(tc.tile_pool(name="wt", bufs=1))
            yhp = ctx.enter_context(tc.tile_pool(name="yhat", bufs=1))
            outp = ctx.enter_context(tc.tile_pool(name="odrain", bufs=6))
            ps_ctx = ExitStack()
            g1ps = ps_ctx.enter_context(tc.tile_pool(name="g1ps", bufs=2, space="PSUM"))
            tps = ps_ctx.enter_context(tc.tile_pool(name="tps", bufs=2, space="PSUM"))

            # ---- constants / small state ----
            ident = small.tile([128, 128], dt.float16, tag="ident")
            make_identity(nc, ident[:])
            ones128 = small.tile([128, 1], dt.float16, tag="ones128")
            nc.vector.memset(ones128[:], 1.0)
            ones1_32 = small.tile([1, 128], dt.float32, tag="ones1_32")
            nc.vector.memset(ones1_32[:], 1.0)
            zeros8 = small.tile([128, CMAX], dt.float32, tag="zeros8")
            nc.vector.memset(zeros8[:], 0.0)
            rm = small.tile([128, M_TILES], dt.float32, tag="rm")
            nc.vector.memset(rm[:], -1e30)
            tmpmax = small.tile([128, 1], dt.float32, tag="tmpmax")
            iota_i = small.tile([128, G], dt.int32, tag="iota_i")
            nc.gpsimd.iota(iota_i[:], pattern=[[128, G]], base=0, channel_multiplier=1)
            iotaF = small.tile([128, G], dt.float32, tag="iotaF")
            nc.vector.tensor_copy(iotaF[:], iota_i[:])
            slot_i = small.tile([128, CMAX], dt.int32, tag="slot_i")
            nc.gpsimd.iota(slot_i[:], pattern=[[128, CMAX]], base=0, channel_multiplier=1)
            slotF = small.tile([128, CMAX], dt.float32, tag="slotF")
            nc.vector.tensor_copy(slotF[:], slot_i[:])

            is2r_t = small.tile([3, 256], dt.float16, tag="is2r")
            nc.sync.dma_start(is2r_t[:], is2r_d)
            y2t_t = small.tile([3, NSP], dt.float16, tag="y2t")
            nc.sync.dma_start(y2t_t[:], y2t_d)
            xst_t = xpool.tile([128, KT, 256], dt.float16, tag="xst")
            nc.sync.dma_start(xst_t[:], xst_d.rearrange("k p b -> p k b"))

            hi16 = hipool.tile([128, G * 256], dt.float16, tag="hi16")

            # ---------------- Phase 1: transposed GEMM1 ----------------
            for ci, (coff, csz) in enumerate(chunks):
                yh = [ypool.tile([128, KH, csz], dt.float16, tag="y", name=f"y{ci}_{h}")
                      for h in range(2)]
                for h in range(2):
                    nc.sync.dma_start(yh[h][:], y_d[ci][h])
                for gi in range(csz // 128):
                    g = coff // 128 + gi
                    ps = g1ps.tile([128, 256], dt.float32, tag="g1ps", name=f"ps{g}")
                    nc.tensor.matmul(ps[:], y2t_t[:, g * 128:(g + 1) * 128],
                                     is2r_t[:], start=True, stop=False)
                    for k in range(KT):
                        nc.tensor.matmul(
                            ps[:],
                            yh[k // KH][:, k % KH, gi * 128:(gi + 1) * 128],
                            xst_t[:, k, :],
                            start=False, stop=(k == KT - 1))
                    hs = hi16[:, g * 256:(g + 1) * 256]
                    nc.scalar.activation(hs, ps[:], Act.Copy)
                    hib = lopool.tile([128, 256], dt.float32, tag="hib", name=f"hib{g}")
                    nc.vector.tensor_copy(hib[:], hs)
                    lo = lopool.tile([128, 256], dt.float16, tag="lo", name=f"lo{g}")
                    nc.vector.tensor_tensor(lo[:], ps[:], hib[:], op=Alu.subtract)
                    nc.sync.dma_start(ynat_d[g * 128:(g + 1) * 128, D:D + 256], hs)
                    nc.sync.dma_start(ynat_d[g * 128:(g + 1) * 128, D + 256:DAUG], lo[:])
                    for m in range(M_TILES):
                        tp = tps.tile([128, 128], dt.float16, tag="tp", name=f"tp{g}_{m}")
                        nc.tensor.matmul(tp[:], hi16[:, g * 256 + m * 128:g * 256 + (m + 1) * 128],
                                         ident[:], is_transpose=True, start=True, stop=True)
                        nc.vector.reduce_max(tmpmax[:], tp[:], mybir.AxisListType.X)
                        nc.vector.tensor_tensor(rm[:, m:m + 1], rm[:, m:m + 1],
                                                tmpmax[:], op=Alu.max)

            # ---------------- gm broadcast ----------------
            nc.sync.dma_start(mx_d[0].rearrange("m p -> p m"), rm[:])
            gms_t = small.tile([1, 256], dt.float32, tag="gms")
            nc.sync.dma_start(gms_t[:], mx_d.rearrange("o m p -> o (m p)"))
            bps = g1ps.tile([128, 256], dt.float32, tag="g1ps", name="bps")
            nc.tensor.matmul(bps[:], ones1_32[:], gms_t[:], start=True, stop=True)
            gmB32 = small.tile([128, 256], dt.float32, tag="gmB32")
            nc.vector.tensor_copy(gmB32[:], bps[:])
            gmB16 = small.tile([128, 256], dt.float16, tag="gmB16")
            nc.scalar.activation(gmB16[:], bps[:], Act.Copy)

            ps_ctx.close()   # free phase-1 PSUM banks
            g2ps = ctx.enter_context(tc.tile_pool(name="g2ps", bufs=6, space="PSUM"))
            mps = ctx.enter_context(tc.tile_pool(name="mps", bufs=1, space="PSUM"))

            # ---------------- selection ----------------
            keep = small.tile([128, G], dt.float32, tag="keep")
            for g in range(G):
                s = spool.tile([128, 256], dt.float16, tag="s", name=f"s{g}")
                nc.vector.tensor_tensor(s[:], hi16[:, g * 256:(g + 1) * 256],
                                        gmB16[:], op=Alu.subtract)
                nc.vector.reduce_max(keep[:, g:g + 1], s[:], mybir.AxisListType.X)

            maskv = small.tile([128, G], dt.float32, tag="maskv")
            nc.vector.tensor_scalar(maskv[:], keep[:], LNTAU, None, op0=Alu.is_ge)
            valsA = small.tile([128, G], dt.float32, tag="valsA")
            nc.vector.tensor_scalar(valsA[:], iotaF[:], 1.0, None, op0=Alu.add)
            valsB = small.tile([128, G], dt.float32, tag="valsB")
            nc.vector.tensor_tensor(valsB[:], valsA[:], maskv[:], op=Alu.mult)
            valsC = small.tile([128, G], dt.float32, tag="valsC")
            nc.vector.tensor_scalar(valsC[:], valsB[:], -1.0, None, op0=Alu.add)
            nc.sync.dma_start(scr1_d.rearrange("g p -> p g"), valsC[:])
            vals16 = small.tile([16, G, 8], dt.float32, tag="vals16")
            nc.sync.dma_start(vals16[:], scr1_d.rearrange("g (pf pp) -> pp g pf", pp=16))
            spout = small.tile([16, CMAX * 8], dt.float32, tag="spout")
            nf = small.tile([1, 1], dt.uint32, tag="nf")
            nc.gpsimd.sparse_gather(spout[:], vals16[:].rearrange("a b c -> a (b c)"),
                                    num_found=nf[:])

            # ---------------- slot index/validity ----------------
            nc.sync.dma_start(scr2_d.rearrange("f plo -> plo f"), spout[:])
            idxf = small.tile([128, CMAX], dt.float32, tag="idxf")
            nc.sync.dma_start(idxf[:], scr2_d.rearrange("(j pf) plo -> (pf plo) j", j=CMAX))
            nf32 = small.tile([1, 1], dt.float32, tag="nf32")
            nc.vector.tensor_copy(nf32[:], nf[:])
            nfp = mps.tile([128, 1], dt.float32, tag="nfp")
            nc.tensor.matmul(nfp[:], ones1_32[:], nf32[:], start=True, stop=True)
            nfbs = small.tile([128, 1], dt.float32, tag="nfbs")
            nc.vector.tensor_copy(nfbs[:], nfp[:])
            mask8 = small.tile([128, CMAX], dt.float32, tag="mask8")
            nc.vector.tensor_scalar(mask8[:], slotF[:], nfbs[:, 0:1], None, op0=Alu.is_lt)
            mm8 = small.tile([128, CMAX], dt.float32, tag="mm8")
            nc.vector.tensor_scalar(mm8[:], mask8[:], 1e30, -1e30, op0=Alu.mult, op1=Alu.add)
            idx32a = small.tile([128, CMAX], dt.int32, tag="idx32a")
            nc.vector.tensor_copy(idx32a[:], idxf[:])
            idx32b = small.tile([128, CMAX], dt.int32, tag="idx32b")
            nc.vector.tensor_scalar(idx32b[:], idx32a[:], NSP - 1, None, op0=Alu.min)
            idx32 = small.tile([128, CMAX], dt.int32, tag="idx32")
            nc.vector.tensor_scalar(idx32[:], idx32b[:], 0, None, op0=Alu.max)

            # ---------------- gather + W' + den ----------------
            yhat = yhp.tile([128, CMAX, DAUG], dt.float16, tag="yhat")
            wt = wtp.tile([128, CMAX, 256], dt.float16, tag="wt")
            denp = mps.tile([1, 256], dt.float32, tag="denp")
            for j in range(CMAX):
                nc.gpsimd.indirect_dma_start(
                    out=yhat[:, j, :], out_offset=None, in_=ynat_d,
                    in_offset=bass.IndirectOffsetOnAxis(ap=idx32[:, j:j + 1], axis=0),
                    bounds_check=NSP - 1, oob_is_err=False)
                ta = tpool.tile([128, 256], dt.float32, tag="ta", name=f"ta{j}")
                nc.vector.tensor_tensor(ta[:], yhat[:, j, D:D + 256],
                                        yhat[:, j, D + 256:DAUG], op=Alu.add)
                tb = tpool.tile([128, 256], dt.float32, tag="tb", name=f"tb{j}")
                nc.vector.tensor_tensor(tb[:], ta[:], gmB32[:], op=Alu.subtract)
                nc.vector.tensor_scalar(tb[:], tb[:], mm8[:, j:j + 1], None, op0=Alu.add)
                nc.scalar.activation(wt[:, j, :], tb[:], Act.Exp)
                nc.tensor.matmul(denp[:], ones128[:], wt[:, j, :],
                                 start=(j == 0), stop=(j == CMAX - 1))

            den_s = small.tile([1, 256], dt.float32, tag="den_s")
            nc.vector.tensor_copy(den_s[:], denp[:])
            nc.sync.dma_start(den_d, den_s[:])

            # ---------------- GEMM2 (m-blocked, j-pipelined) ----------------
            for m in range(M_TILES):
                ps6 = [g2ps.tile([128, 512], dt.float32, tag="g2ps", name=f"g2_{m}_{s}")
                       for s in range(6)]
                for j in range(CMAX):
                    for s in range(6):
                        nc.tensor.matmul(
                            ps6[s][:],
                            wt[:, j, m * 128:(m + 1) * 128],
                            yhat[:, j, s * 512:(s + 1) * 512],
                            start=(j == 0), stop=(j == CMAX - 1))
                for s in range(6):
                    o = outp.tile([128, 512], dt.float32, tag="o", name=f"o{m}_{s}")
                    if m == 0:
                        nc.vector.tensor_copy(o[:], ps6[s][:])
                    else:
                        nc.scalar.activation(o[:], ps6[s][:], Act.Copy)
                    nc.sync.dma_start(num_d[m][:, s * 512:(s + 1) * 512], o[:])

    nc.compile()
    return nc


def prep_inputs(input, sigma, train_data, n_cores=N_CORES):
    """Host-side shard + pre-tile. Returns list of per-core in_maps."""
    x = np.asarray(input, dtype=np.float32).reshape(B, D)
    sig = np.asarray(sigma, dtype=np.float64)
    y = np.asarray(train_data, dtype=np.float32).reshape(N_TOTAL, D)

    is2 = (1.0 / sig ** 2).astype(np.float32)                  # [256]
    xt16 = (x * is2[:, None]).astype(np.float16)               # x~ = x*is2
    xst = np.ascontiguousarray(xt16.reshape(B, KT, 128).transpose(1, 2, 0))  # [KT,128,256]
    is2h = is2.astype(np.float16)
    is2l = (is2 - is2h.astype(np.float32)).astype(np.float16)
    is2r = np.stack([is2h, is2h, is2l])                        # [3, 256]

    chunks = chunk_list()
    in_maps = []
    for c in range(n_cores):
        ys = y[c * NS:(c + 1) * NS]
        ys16p = np.zeros((NSP, D), dtype=np.float16)
        ys16p[:NS] = ys.astype(np.float16)
        y2f = (-0.5 * np.einsum("ij,ij->i", ys.astype(np.float64),
                                ys.astype(np.float64))).astype(np.float32)
        y2h = np.full(NSP, PAD_Y2, dtype=np.float16)
        y2l = np.zeros(NSP, dtype=np.float16)
        y2h[:NS] = y2f.astype(np.float16)
        y2l[:NS] = (y2f - y2h[:NS].astype(np.float32)).astype(np.float16)
        y2t = np.stack([y2h, y2l, y2h])                        # [3, NSP]

        ynat = np.zeros((NSP, DAUG), dtype=np.float16)
        ynat[:, :D] = ys16p

        im = {"xst": xst, "is2r": is2r, "y2t": y2t, "ynat": ynat}
        for ci, (coff, csz) in enumerate(chunks):
            yt = ys16p[coff:coff + csz].T.reshape(2, KH, 128, csz)
            im[f"y_c{ci}"] = np.ascontiguousarray(yt.transpose(0, 2, 1, 3))
        in_maps.append(im)
    return in_maps


def combine(results):
    """Flash-style combine of per-core (num, den, mx) partials -> full output."""
    num = np.stack([r["num"].reshape(B, D) for r in results]).astype(np.float64)
    den = np.stack([r["den"].reshape(B) for r in results]).astype(np.float64)
    mx = np.stack([r["mx"].reshape(B) for r in results]).astype(np.float64)
    M = mx.max(axis=0)
    r = np.exp(mx - M[None, :])
    num_tot = (num * r[:, :, None]).sum(axis=0)
    den_tot = (den * r).sum(axis=0)
    out = (num_tot / den_tot[:, None]).astype(np.float32)
    return out.reshape(B, C, H, W_IMG)


_NC_CACHE = {}


def get_nc():
    if "nc" not in _NC_CACHE:
        _NC_CACHE["nc"] = build_nc()
    return _NC_CACHE["nc"]


def kernel(input, sigma, train_data):
    nc = get_nc()
    in_maps = prep_inputs(input, sigma, train_data)
    res = run_bass_kernel_spmd(nc, in_maps, core_ids=list(range(N_CORES)))
    return combine(res.results)


# revision 19
# speedup vs baseline: 1.1150x; 1.1150x over previous
"""Karras optimal denoiser on 8 Trainium2 NeuronCores — sparse-GEMM2 version.

Math: out_b = sum_i w_bi y_i / sum_i w_bi,  w_bi = exp((x_b.y_i - 0.5||y_i||^2)/sigma_b^2).
Softmax over N=50000 is extremely peaked (retrieval-knn): per core only ~700 of
6250 columns have weight mass >= tau=exp(-14) relative to the local row max.

Per-core plan (train_data sharded over N, flash-style host combine):
  Phase 1 (GEMM1, transposed): LamT[i, b] = (x_b*is2_b . y_i) + (-0.5||y_i||^2)*is2_b
    computed 128-column-group at a time: stationary = y^T k-tiles, moving = x~^T.
    The y2*is2 term enters as a K=3 matmul (hi/lo fp16 split of both factors).
    Per group: PSUM -> hi16 (fp16) + lo16 (fp16 residual, exact to ~2^-12),
    both DMA'd into spare columns of the ynat DRAM tensor (rows = train points);
    hi16 also kept in SBUF. Row maxes via 2 PE transposes + DVE reduce_max.
  Selection: keepscore_i = max_b (hi - gm~_b); columns with keepscore >= ln(tau)
    are compacted to a rank list by gpsimd sparse_gather ([16,F] wrapped layout;
    DRAM-bounce rearranges to/from it). Slots are rank-ordered => perfectly
    partition-balanced. Slot validity from num_found (HW tail is garbage).
  Gather: per slot-block j, one indirect DMA fetches 128 rows of
    ynat = [y row fp16 (3072) | hi (256) | lo (256)] -> 7KB rows at ~line rate.
  W'^T_j = exp(hi + lo - gm~) (invalid slots killed via -1e30 bias); den = ones^T W'.
  GEMM2: num = W'^T.T @ yhat over CMAX j-blocks, m-blocked (6 PSUM banks),
    pipelined against the gathers.
Outputs per core: num [2,128,3072], den [1,256], mx [128,2] (=gm~, scaled-logit
frame); host does the flash combine in that frame.
"""

import numpy as np
from contextlib import ExitStack

import concourse.bass as bass
import concourse.tile as tile
import concourse.mybir as mybir
from concourse import bacc
from concourse.bass_utils import run_bass_kernel_spmd
from concourse.masks import make_identity

dt = mybir.dt
Alu = mybir.AluOpType
Act = mybir.ActivationFunctionType

B, C, H, W_IMG = 256, 3, 32, 32
D = C * H * W_IMG            # 3072
N_TOTAL = 50000
N_CORES = 8
NS = N_TOTAL // N_CORES      # 6250 per core
NSP = 6272                   # padded to 49*128
KT = D // 128                # 24 contraction k-tiles
KH = KT // 2                 # 12 per stream half
G = NSP // 128               # 49 column groups
M_TILES = 2                  # 256 query rows = 2 partition tiles
CMAX = 8                     # survivor slot blocks (cap = 128*CMAX = 1024)
LNTAU = -14.0                # keep threshold on log-weights
DAUG = D + 512               # ynat row: y | hi | lo
PAD_Y2 = -15000.0            # pad-row -0.5||y||^2 sentinel (never selected; *4 stays in fp16)


def chunk_list(nsp=NSP):
    """128-multiple chunks, ramped so the first y DMAs land fast."""
    sizes = [128, 256, 384]
    out, off = [], 0
    for s in sizes:
        out.append((off, s))
        off += s
    while off < nsp:
        out.append((off, min(512, nsp - off)))
        off += 512
    return out


def build_nc():
    chunks = chunk_list()
    nc = bacc.Bacc("TRN2", target_bir_lowering=False, debug=False)

    # --- DRAM I/O ---
    y_d = [nc.dram_tensor(f"y_c{ci}", (2, 128, KH, csz), dt.float16, kind="ExternalInput").ap()
           for ci, (_, csz) in enumerate(chunks)]
    y2t_d = nc.dram_tensor("y2t", (3, NSP), dt.float16, kind="ExternalInput").ap()
    is2r_d = nc.dram_tensor("is2r", (3, 256), dt.float16, kind="ExternalInput").ap()
    xst_d = nc.dram_tensor("xst", (KT, 128, 256), dt.float16, kind="ExternalInput").ap()
    ynat_d = nc.dram_tensor("ynat", (NSP, DAUG), dt.float16, kind="ExternalInput").ap()

    num_d = nc.dram_tensor("num", (M_TILES, 128, D), dt.float32, kind="ExternalOutput").ap()
    den_d = nc.dram_tensor("den", (1, 256), dt.float32, kind="ExternalOutput").ap()
    mx_d = nc.dram_tensor("mx", (1, M_TILES, 128), dt.float32, kind="ExternalOutput").ap()
    scr1_d = nc.dram_tensor("scr1", (G, 128), dt.float32, kind="ExternalOutput").ap()
    scr2_d = nc.dram_tensor("scr2", (CMAX * 8, 16), dt.float32, kind="ExternalOutput").ap()

    with tile.TileContext(nc) as tc:
        with ExitStack() as ctx:
            small = ctx.enter_context(tc.tile_pool(name="small", bufs=1))
            xpool = ctx.enter_context(tc.tile_pool(name="x", bufs=1))
            ypool = ctx.enter_context(tc.tile_pool(name="ystream", bufs=4))
            hipool = ctx.enter_context(tc.tile_pool(name="hi", bufs=1))
            lopool = ctx.enter_context(tc.tile_pool(name="lo", bufs=3))
            spool = ctx.enter_context(tc.tile_pool(name="ssel", bufs=4))
            tpool = ctx.enter_context(tc.tile_pool(name="texp", bufs=3))
            wtp = ctx.enter_context(tc.tile_pool(name="wt", bufs=1))
            yhp = ctx.enter_context(tc.tile_pool(name="yhat", bufs=1))
            outp = ctx.enter_context(tc.tile_pool(name="odrain", bufs=4))
            ps_ctx = ExitStack()
            g1ps = ps_ctx.enter_context(tc.tile_pool(name="g1ps", bufs=2, space="PSUM"))
            tps = ps_ctx.enter_context(tc.tile_pool(name="tps", bufs=2, space="PSUM"))

            # ---- constants / small state ----
            ident = small.tile([128, 128], dt.float16, tag="ident")
            make_identity(nc, ident[:])
            ones128 = small.tile([128, 1], dt.float16, tag="ones128")
            nc.vector.memset(ones128[:], 1.0)
            ones1_32 = small.tile([1, 128], dt.float32, tag="ones1_32")
            nc.vector.memset(ones1_32[:], 1.0)
            zeros8 = small.tile([128, CMAX], dt.float32, tag="zeros8")
            nc.vector.memset(zeros8[:], 0.0)
            rm = small.tile([128, M_TILES], dt.float32, tag="rm")
            colmax = small.tile([128, M_TILES * G], dt.float32, tag="colmax")
            iota_i = small.tile([128, G], dt.int32, tag="iota_i")
            nc.gpsimd.iota(iota_i[:], pattern=[[128, G]], base=0, channel_multiplier=1)
            iotaF = small.tile([128, G], dt.float32, tag="iotaF")
            nc.vector.tensor_copy(iotaF[:], iota_i[:])
            slot_i = small.tile([128, CMAX], dt.int32, tag="slot_i")
            nc.gpsimd.iota(slot_i[:], pattern=[[128, CMAX]], base=0, channel_multiplier=1)
            slotF = small.tile([128, CMAX], dt.float32, tag="slotF")
            nc.vector.tensor_copy(slotF[:], slot_i[:])

            is2r_t = small.tile([3, 256], dt.float16, tag="is2r")
            nc.sync.dma_start(is2r_t[:], is2r_d)
            y2t_t = small.tile([3, NSP], dt.float16, tag="y2t")
            nc.sync.dma_start(y2t_t[:], y2t_d)
            xst_t = xpool.tile([128, KT, 256], dt.float16, tag="xst")
            nc.sync.dma_start(xst_t[:], xst_d.rearrange("k p b -> p k b"))

            # hilo[:, g*512 : g*512+256] = hi (fp16 of LamT), [+256:+512] = lo residual
            hilo = hipool.tile([128, G * 512], dt.float16, tag="hilo")

            def emit_transposes(g):
                """PE transposes of group g's hi (for row maxes) — emitted a few
                groups late so PE never waits on the ACT hi-copy."""
                for m in range(M_TILES):
                    tp = tps.tile([128, 128], dt.float16, tag="tp", name=f"tp{g}_{m}")
                    nc.tensor.matmul(tp[:], hilo[:, g * 512 + m * 128:g * 512 + (m + 1) * 128],
                                     ident[:], is_transpose=True, start=True, stop=True)
                    nc.vector.reduce_max(colmax[:, m * G + g:m * G + g + 1], tp[:],
                                         mybir.AxisListType.X)

            # ---------------- Phase 1: transposed GEMM1 ----------------
            TDELAY = 2
            for ci, (coff, csz) in enumerate(chunks):
                yh = [ypool.tile([128, KH, csz], dt.float16, tag="y", name=f"y{ci}_{h}")
                      for h in range(2)]
                for h in range(2):
                    nc.sync.dma_start(yh[h][:], y_d[ci][h])
                for gi in range(csz // 128):
                    g = coff // 128 + gi
                    ps = g1ps.tile([128, 256], dt.float32, tag="g1ps", name=f"ps{g}")
                    nc.tensor.matmul(ps[:], y2t_t[:, g * 128:(g + 1) * 128],
                                     is2r_t[:], start=True, stop=False)
                    for k in range(KT):
                        nc.tensor.matmul(
                            ps[:],
                            yh[k // KH][:, k % KH, gi * 128:(gi + 1) * 128],
                            xst_t[:, k, :],
                            start=False, stop=(k == KT - 1))
                    hs = hilo[:, g * 512:g * 512 + 256]
                    nc.scalar.activation(hs, ps[:], Act.Copy)
                    lo = hilo[:, g * 512 + 256:g * 512 + 512]
                    nc.vector.tensor_tensor(lo, ps[:], hs, op=Alu.subtract)
                    nc.sync.dma_start(ynat_d[g * 128:(g + 1) * 128, D:DAUG],
                                      hilo[:, g * 512:(g + 1) * 512])
                    if g >= TDELAY:
                        emit_transposes(g - TDELAY)
            for g in range(G - TDELAY, G):
                emit_transposes(g)
            for m in range(M_TILES):
                nc.vector.reduce_max(rm[:, m:m + 1], colmax[:, m * G:(m + 1) * G],
                                     mybir.AxisListType.X)

            # ---------------- gm broadcast ----------------
            nc.sync.dma_start(mx_d[0].rearrange("m p -> p m"), rm[:])
            gms_t = small.tile([1, 256], dt.float32, tag="gms")
            nc.sync.dma_start(gms_t[:], mx_d.rearrange("o m p -> o (m p)"))
            bps = g1ps.tile([128, 256], dt.float32, tag="g1ps", name="bps")
            nc.tensor.matmul(bps[:], ones1_32[:], gms_t[:], start=True, stop=True)
            gmB32 = small.tile([128, 256], dt.float32, tag="gmB32")
            nc.vector.tensor_copy(gmB32[:], bps[:])
            gmB16 = small.tile([128, 256], dt.float16, tag="gmB16")
            nc.scalar.activation(gmB16[:], bps[:], Act.Copy)

            ps_ctx.close()   # free phase-1 PSUM banks
            g2ps = ctx.enter_context(tc.tile_pool(name="g2ps", bufs=6, space="PSUM"))
            mps = ctx.enter_context(tc.tile_pool(name="mps", bufs=1, space="PSUM"))

            # ---------------- selection ----------------
            keep = small.tile([128, G], dt.float32, tag="keep")
            for g in range(G):
                s = spool.tile([128, 256], dt.float16, tag="s", name=f"s{g}")
                nc.vector.tensor_tensor(s[:], hilo[:, g * 512:g * 512 + 256],
                                        gmB16[:], op=Alu.subtract)
                nc.vector.reduce_max(keep[:, g:g + 1], s[:], mybir.AxisListType.X)

            maskv = small.tile([128, G], dt.float32, tag="maskv")
            nc.vector.tensor_scalar(maskv[:], keep[:], LNTAU, None, op0=Alu.is_ge)
            valsA = small.tile([128, G], dt.float32, tag="valsA")
            nc.vector.tensor_scalar(valsA[:], iotaF[:], 1.0, None, op0=Alu.add)
            valsB = small.tile([128, G], dt.float32, tag="valsB")
            nc.vector.tensor_tensor(valsB[:], valsA[:], maskv[:], op=Alu.mult)
            valsC = small.tile([128, G], dt.float32, tag="valsC")
            nc.vector.tensor_scalar(valsC[:], valsB[:], -1.0, None, op0=Alu.add)
            nc.sync.dma_start(scr1_d.rearrange("g p -> p g"), valsC[:])
            vals16 = small.tile([16, G, 8], dt.float32, tag="vals16")
            nc.sync.dma_start(vals16[:], scr1_d.rearrange("g (pf pp) -> pp g pf", pp=16))
            spout = small.tile([16, CMAX * 8], dt.float32, tag="spout")
            nf = small.tile([1, 1], dt.uint32, tag="nf")
            nc.gpsimd.sparse_gather(spout[:], vals16[:].rearrange("a b c -> a (b c)"),
                                    num_found=nf[:])

            # ---------------- slot index/validity ----------------
            nc.sync.dma_start(scr2_d.rearrange("f plo -> plo f"), spout[:])
            idxf = small.tile([128, CMAX], dt.float32, tag="idxf")
            nc.sync.dma_start(idxf[:], scr2_d.rearrange("(j pf) plo -> (pf plo) j", j=CMAX))
            nf32 = small.tile([1, 1], dt.float32, tag="nf32")
            nc.vector.tensor_copy(nf32[:], nf[:])
            nfp = mps.tile([128, 1], dt.float32, tag="nfp")
            nc.tensor.matmul(nfp[:], ones1_32[:], nf32[:], start=True, stop=True)
            nfbs = small.tile([128, 1], dt.float32, tag="nfbs")
            nc.vector.tensor_copy(nfbs[:], nfp[:])
            mask8 = small.tile([128, CMAX], dt.float32, tag="mask8")
            nc.vector.tensor_scalar(mask8[:], slotF[:], nfbs[:, 0:1], None, op0=Alu.is_lt)
            mm8 = small.tile([128, CMAX], dt.float32, tag="mm8")
            nc.vector.tensor_scalar(mm8[:], mask8[:], 1e30, -1e30, op0=Alu.mult, op1=Alu.add)
            idx32a = small.tile([128, CMAX], dt.int32, tag="idx32a")
            nc.vector.tensor_copy(idx32a[:], idxf[:])
            idx32b = small.tile([128, CMAX], dt.int32, tag="idx32b")
            nc.vector.tensor_scalar(idx32b[:], idx32a[:], NSP - 1, None, op0=Alu.min)
            idx32 = small.tile([128, CMAX], dt.int32, tag="idx32")
            nc.vector.tensor_scalar(idx32[:], idx32b[:], 0, None, op0=Alu.max)

            # ---------------- gather + W' + den ----------------
            yhat = yhp.tile([128, CMAX, DAUG], dt.float16, tag="yhat")
            wt = wtp.tile([128, CMAX, 256], dt.float16, tag="wt")
            denp = mps.tile([1, 256], dt.float32, tag="denp")
            for j in range(CMAX):
                nc.gpsimd.indirect_dma_start(
                    out=yhat[:, j, :], out_offset=None, in_=ynat_d,
                    in_offset=bass.IndirectOffsetOnAxis(ap=idx32[:, j:j + 1], axis=0),
                    bounds_check=NSP - 1, oob_is_err=False)
                ta = tpool.tile([128, 256], dt.float32, tag="ta", name=f"ta{j}")
                nc.vector.tensor_tensor(ta[:], yhat[:, j, D:D + 256],
                                        yhat[:, j, D + 256:DAUG], op=Alu.add)
                tb = tpool.tile([128, 256], dt.float32, tag="tb", name=f"tb{j}")
                nc.vector.tensor_tensor(tb[:], ta[:], gmB32[:], op=Alu.subtract)
                nc.vector.tensor_scalar(tb[:], tb[:], mm8[:, j:j + 1], None, op0=Alu.add)
                nc.scalar.activation(wt[:, j, :], tb[:], Act.Exp)
                nc.tensor.matmul(denp[:], ones128[:], wt[:, j, :],
                                 start=(j == 0), stop=(j == CMAX - 1))

            den_s = small.tile([1, 256], dt.float32, tag="den_s")
            nc.vector.tensor_copy(den_s[:], denp[:])
            nc.sync.dma_start(den_d, den_s[:])

            # ---------------- GEMM2 (m-blocked, j-pipelined) ----------------
            for m in range(M_TILES):
                ps6 = [g2ps.tile([128, 512], dt.float32, tag="g2ps", name=f"g2_{m}_{s}")
                       for s in range(6)]
                for j in range(CMAX):
                    for s in range(6):
                        nc.tensor.matmul(
                            ps6[s][:],
                            wt[:, j, m * 128:(m + 1) * 128],
                            yhat[:, j, s * 512:(s + 1) * 512],
                            start=(j == 0), stop=(j == CMAX - 1))
                for s in range(6):
                    o = outp.tile([128, 512], dt.float32, tag="o", name=f"o{m}_{s}")
                    if m == 0:
                        nc.vector.tensor_copy(o[:], ps6[s][:])
                    else:
                        nc.scalar.activation(o[:], ps6[s][:], Act.Copy)
                    nc.sync.dma_start(num_d[m][:, s * 512:(s + 1) * 512], o[:])

    nc.compile()
    return nc


def prep_inputs(input, sigma, train_data, n_cores=N_CORES):
    """Host-side shard + pre-tile. Returns list of per-core in_maps."""
    x = np.asarray(input, dtype=np.float32).reshape(B, D)
    sig = np.asarray(sigma, dtype=np.float64)
    y = np.asarray(train_data, dtype=np.float32).reshape(N_TOTAL, D)

    is2 = (1.0 / sig ** 2).astype(np.float32)                  # [256]
    xt16 = (x * is2[:, None]).astype(np.float16)               # x~ = x*is2
    xst = np.ascontiguousarray(xt16.reshape(B, KT, 128).transpose(1, 2, 0))  # [KT,128,256]
    is2h = is2.astype(np.float16)
    is2l = (is2 - is2h.astype(np.float32)).astype(np.float16)
    is2r = np.stack([is2h, is2h, is2l])                        # [3, 256]

    chunks = chunk_list()
    in_maps = []
    for c in range(n_cores):
        ys = y[c * NS:(c + 1) * NS]
        ys16p = np.zeros((NSP, D), dtype=np.float16)
        ys16p[:NS] = ys.astype(np.float16)
        y2f = (-0.5 * np.einsum("ij,ij->i", ys.astype(np.float64),
                                ys.astype(np.float64))).astype(np.float32)
        y2h = np.full(NSP, PAD_Y2, dtype=np.float16)
        y2l = np.zeros(NSP, dtype=np.float16)
        y2h[:NS] = y2f.astype(np.float16)
        y2l[:NS] = (y2f - y2h[:NS].astype(np.float32)).astype(np.float16)
        y2t = np.stack([y2h, y2l, y2h])                        # [3, NSP]

        ynat = np.zeros((NSP, DAUG), dtype=np.float16)
        ynat[:, :D] = ys16p

        im = {"xst": xst, "is2r": is2r, "y2t": y2t, "ynat": ynat}
        for ci, (coff, csz) in enumerate(chunks):
            yt = ys16p[coff:coff + csz].T.reshape(2, KH, 128, csz)
            im[f"y_c{ci}"] = np.ascontiguousarray(yt.transpose(0, 2, 1, 3))
        in_maps.append(im)
    return in_maps


def combine(results):
    """Flash-style combine of per-core (num, den, mx) partials -> full output."""
    num = np.stack([r["num"].reshape(B, D) for r in results]).astype(np.float64)
    den = np.stack([r["den"].reshape(B) for r in results]).astype(np.float64)
    mx = np.stack([r["mx"].reshape(B) for r in results]).astype(np.float64)
    M = mx.max(axis=0)
    r = np.exp(mx - M[None, :])
    num_tot = (num * r[:, :, None]).sum(axis=0)
    den_tot = (den * r).sum(axis=0)
    out = (num_tot / den_tot[:, None]).astype(np.float32)
    return out.reshape(B, C, H, W_IMG)


_NC_CACHE = {}


def get_nc():
    if "nc" not in _NC_CACHE:
        _NC_CACHE["nc"] = build_nc()
    return _NC_CACHE["nc"]


def kernel(input, sigma, train_data):
    nc = get_nc()
    in_maps = prep_inputs(input, sigma, train_data)
    res = run_bass_kernel_spmd(nc, in_maps, core_ids=list(range(N_CORES)))
    return combine(res.results)


# revision 31
# speedup vs baseline: 1.2414x; 1.1134x over previous
"""Karras optimal denoiser on 8 Trainium2 NeuronCores — sparse-GEMM2 version.

Math: out_b = sum_i w_bi y_i / sum_i w_bi,  w_bi = exp((x_b.y_i - 0.5||y_i||^2)/sigma_b^2).
Softmax over N=50000 is extremely peaked (retrieval-knn): per core only ~700 of
6250 columns have weight mass >= tau=exp(-14) relative to the local row max.

Per-core plan (train_data sharded over N, flash-style host combine):
  Phase 1 (GEMM1, transposed): LamT[i, b] = (x_b*is2_b . y_i) + (-0.5||y_i||^2)*is2_b
    computed 128-column-group at a time: stationary = y^T k-tiles, moving = x~^T.
    The y2*is2 term enters as a K=3 matmul (hi/lo fp16 split of both factors).
    Per group: PSUM -> hi16 (fp16) + lo16 (fp16 residual, exact to ~2^-12),
    both DMA'd into spare columns of the ynat DRAM tensor (rows = train points);
    hi16 also kept in SBUF. Row maxes via 2 PE transposes + DVE reduce_max.
  Selection: keepscore_i = max_b (hi - gm~_b); columns with keepscore >= ln(tau)
    are compacted to a rank list by gpsimd sparse_gather ([16,F] wrapped layout;
    DRAM-bounce rearranges to/from it). Slots are rank-ordered => perfectly
    partition-balanced. Slot validity from num_found (HW tail is garbage).
  Gather: per slot-block j, one indirect DMA fetches 128 rows of
    ynat = [y row fp16 (3072) | hi (256) | lo (256)] -> 7KB rows at ~line rate.
  W'^T_j = exp(hi + lo - gm~) (invalid slots killed via -1e30 bias); den = ones^T W'.
  GEMM2: num = W'^T.T @ yhat over CMAX j-blocks, m-blocked (6 PSUM banks),
    pipelined against the gathers.
Outputs per core: num [2,128,3072], den [1,256], mx [128,2] (=gm~, scaled-logit
frame); host does the flash combine in that frame.
"""

import numpy as np
from contextlib import ExitStack

import concourse.bass as bass
import concourse.tile as tile
import concourse.mybir as mybir
from concourse import bacc
from concourse.bass_utils import run_bass_kernel_spmd
from concourse.masks import make_identity

dt = mybir.dt
Alu = mybir.AluOpType
Act = mybir.ActivationFunctionType

B, C, H, W_IMG = 256, 3, 32, 32
D = C * H * W_IMG            # 3072
N_TOTAL = 50000
N_CORES = 8
NS = N_TOTAL // N_CORES      # 6250 per core
NSP = 6272                   # padded to 49*128
KT = D // 128                # 24 contraction k-tiles
KH = KT // 2                 # 12 per stream half
G = NSP // 128               # 49 column groups
M_TILES = 2                  # 256 query rows = 2 partition tiles
CMAX = 8                     # survivor slot blocks (cap = 128*CMAX = 1024)
LNTAU = -14.0                # keep threshold on log-weights
DAUG = D + 512               # ynat row: y | hi | lo
PAD_Y2 = -15000.0            # pad-row -0.5||y||^2 sentinel (never selected; *4 stays in fp16)


def chunk_list(nsp=NSP):
    """128-multiple chunks, ramped so the first y DMAs land fast."""
    sizes = [128, 256, 384]
    out, off = [], 0
    for s in sizes:
        out.append((off, s))
        off += s
    while off < nsp:
        out.append((off, min(512, nsp - off)))
        off += 512
    return out


def build_nc():
    chunks = chunk_list()
    nc = bacc.Bacc("TRN2", target_bir_lowering=False, debug=False)

    # --- DRAM I/O ---
    y_d = [nc.dram_tensor(f"y_c{ci}", (2, 128, KH, csz), dt.float16, kind="ExternalInput").ap()
           for ci, (_, csz) in enumerate(chunks)]
    y2t_d = nc.dram_tensor("y2t", (3, NSP), dt.float16, kind="ExternalInput").ap()
    is2r_d = nc.dram_tensor("is2r", (3, 256), dt.float16, kind="ExternalInput").ap()
    xst_d = nc.dram_tensor("xst", (KT, 128, 256), dt.float16, kind="ExternalInput").ap()
    ynat_d = nc.dram_tensor("ynat", (NSP, DAUG), dt.float16, kind="ExternalInput").ap()

    num_d = nc.dram_tensor("num", (M_TILES, 128, D), dt.float32, kind="ExternalOutput").ap()
    den_d = nc.dram_tensor("den", (1, 256), dt.float32, kind="ExternalOutput").ap()
    slotr_d = nc.dram_tensor("slotr", (128, CMAX), dt.int32, kind="ExternalInput").ap()
    onesel_d = nc.dram_tensor("onesel", (2, 256), dt.float32, kind="ExternalInput").ap()
    mx_d = nc.dram_tensor("mx", (1, 128, M_TILES), dt.float32, kind="ExternalOutput").ap()
    scr1_d = nc.dram_tensor("scr1", (128, G), dt.float32, kind="ExternalOutput").ap()
    scr2_d = nc.dram_tensor("scr2", (16, CMAX * 8), dt.float32, kind="ExternalOutput").ap()

    with tile.TileContext(nc) as tc:
        with ExitStack() as ctx:
            small = ctx.enter_context(tc.tile_pool(name="small", bufs=1))
            xpool = ctx.enter_context(tc.tile_pool(name="x", bufs=1))
            ypool = ctx.enter_context(tc.tile_pool(name="ystream", bufs=4))
            hipool = ctx.enter_context(tc.tile_pool(name="hi", bufs=1))
            lopool = ctx.enter_context(tc.tile_pool(name="lo", bufs=3))
            spool = ctx.enter_context(tc.tile_pool(name="ssel", bufs=4))
            tpool = ctx.enter_context(tc.tile_pool(name="texp", bufs=3))
            wtp = ctx.enter_context(tc.tile_pool(name="wt", bufs=1))
            yhp = ctx.enter_context(tc.tile_pool(name="yhat", bufs=1))
            outp = ctx.enter_context(tc.tile_pool(name="odrain", bufs=4))
            ps_ctx = ExitStack()
            g1ps = ps_ctx.enter_context(tc.tile_pool(name="g1ps", bufs=2, space="PSUM"))
            tps = ps_ctx.enter_context(tc.tile_pool(name="tps", bufs=2, space="PSUM"))

            # ---- constants / small state ----
            ident = small.tile([128, 128], dt.float16, tag="ident")
            make_identity(nc, ident[:])
            ident32 = small.tile([128, 128], dt.float32, tag="ident32")
            nc.vector.tensor_copy(ident32[:], ident[:])
            ones128 = small.tile([128, 1], dt.float16, tag="ones128")
            nc.vector.memset(ones128[:], 1.0)
            ones1_32 = small.tile([1, 128], dt.float32, tag="ones1_32")
            nc.vector.memset(ones1_32[:], 1.0)
            zeros8 = small.tile([128, CMAX], dt.float32, tag="zeros8")
            nc.vector.memset(zeros8[:], 0.0)
            rm = small.tile([128, M_TILES], dt.float32, tag="rm")
            colmax = small.tile([128, M_TILES * G], dt.float32, tag="colmax")
            iota_i = small.tile([128, G], dt.int32, tag="iota_i")
            nc.gpsimd.iota(iota_i[:], pattern=[[128, G]], base=0, channel_multiplier=1)
            iotaF = small.tile([128, G], dt.float32, tag="iotaF")
            nc.vector.tensor_copy(iotaF[:], iota_i[:])
            slot_i = small.tile([128, CMAX], dt.int32, tag="slot_i")
            nc.sync.dma_start(slot_i[:], slotr_d)
            slotF = small.tile([128, CMAX], dt.float32, tag="slotF")
            nc.vector.tensor_copy(slotF[:], slot_i[:])
            onesel = small.tile([2, 256], dt.float32, tag="onesel")
            nc.sync.dma_start(onesel[:], onesel_d)

            is2r_t = small.tile([3, 256], dt.float16, tag="is2r")
            nc.sync.dma_start(is2r_t[:], is2r_d)
            y2t_t = small.tile([3, NSP], dt.float16, tag="y2t")
            nc.sync.dma_start(y2t_t[:], y2t_d)
            xst_t = xpool.tile([128, KT, 256], dt.float16, tag="xst")
            nc.sync.dma_start(xst_t[:], xst_d.rearrange("k p b -> p k b"))

            # hilo[:, g*512 : g*512+256] = hi (fp16 of LamT), [+256:+512] = lo residual
            hilo = hipool.tile([128, G * 512], dt.float16, tag="hilo")

            def emit_transposes(g):
                """PE transposes of group g's hi (for row maxes) — emitted a few
                groups late so PE never waits on the ACT hi-copy."""
                for m in range(M_TILES):
                    tp = tps.tile([128, 128], dt.float16, tag="tp", name=f"tp{g}_{m}")
                    nc.tensor.matmul(tp[:], hilo[:, g * 512 + m * 128:g * 512 + (m + 1) * 128],
                                     ident[:], is_transpose=True, start=True, stop=True)
                    nc.vector.reduce_max(colmax[:, m * G + g:m * G + g + 1], tp[:],
                                         mybir.AxisListType.X)

            # ---------------- Phase 1: transposed GEMM1 ----------------
            TDELAY = 2
            for ci, (coff, csz) in enumerate(chunks):
                yh = [ypool.tile([128, KH, csz], dt.float16, tag="y", name=f"y{ci}_{h}")
                      for h in range(2)]
                for h in range(2):
                    nc.sync.dma_start(yh[h][:], y_d[ci][h])
                for gi in range(csz // 128):
                    g = coff // 128 + gi
                    ps = g1ps.tile([128, 256], dt.float32, tag="g1ps", name=f"ps{g}")
                    nc.tensor.matmul(ps[:], y2t_t[:, g * 128:(g + 1) * 128],
                                     is2r_t[:], start=True, stop=False)
                    for k in range(KT):
                        nc.tensor.matmul(
                            ps[:],
                            yh[k // KH][:, k % KH, gi * 128:(gi + 1) * 128],
                            xst_t[:, k, :],
                            start=False, stop=(k == KT - 1))
                    hs = hilo[:, g * 512:g * 512 + 256]
                    nc.scalar.activation(hs, ps[:], Act.Copy)
                    lo = hilo[:, g * 512 + 256:g * 512 + 512]
                    nc.vector.tensor_tensor(lo, ps[:], hs, op=Alu.subtract)
                    nc.sync.dma_start(ynat_d[g * 128:(g + 1) * 128, D:DAUG],
                                      hilo[:, g * 512:(g + 1) * 512])
                    if g >= TDELAY:
                        emit_transposes(g - TDELAY)
            for g in range(G - TDELAY, G):
                emit_transposes(g)
            for m in range(M_TILES):
                nc.vector.reduce_max(rm[:, m:m + 1], colmax[:, m * G:(m + 1) * G],
                                     mybir.AxisListType.X)

            # ---------------- gm broadcast (PE transpose, no DRAM bounce) ----------------
            nc.sync.dma_start(mx_d[0], rm[:])
            tpr = tps.tile([2, 128], dt.float32, tag="tpr")
            nc.tensor.matmul(tpr[:], rm[:], ident32[:], is_transpose=True, start=True, stop=True)
            gsm2 = small.tile([2, 128], dt.float32, tag="gsm2")
            nc.vector.tensor_copy(gsm2[:], tpr[:])
            bps = g1ps.tile([128, 256], dt.float32, tag="g1ps", name="bps")
            for m in range(M_TILES):
                nc.tensor.matmul(bps[:, m * 128:(m + 1) * 128], onesel[:, m * 128:(m + 1) * 128],
                                 gsm2[:], start=True, stop=True)
            gmB32 = small.tile([128, 256], dt.float32, tag="gmB32")
            nc.vector.tensor_copy(gmB32[:], bps[:])
            gmB16 = small.tile([128, 256], dt.float16, tag="gmB16")
            nc.scalar.activation(gmB16[:], bps[:], Act.Copy)

            ps_ctx.close()   # free phase-1 PSUM banks
            g2ps = ctx.enter_context(tc.tile_pool(name="g2ps", bufs=6, space="PSUM"))
            mps = ctx.enter_context(tc.tile_pool(name="mps", bufs=1, space="PSUM"))

            # ---------------- selection ----------------
            keep = small.tile([128, G], dt.float32, tag="keep")
            for g in range(G):
                s = spool.tile([128, 256], dt.float16, tag="s", name=f"s{g}")
                nc.vector.tensor_tensor(s[:], hilo[:, g * 512:g * 512 + 256],
                                        gmB16[:], op=Alu.subtract)
                nc.vector.reduce_max(keep[:, g:g + 1], s[:], mybir.AxisListType.X)

            maskv = small.tile([128, G], dt.float32, tag="maskv")
            nc.vector.tensor_scalar(maskv[:], keep[:], LNTAU, None, op0=Alu.is_ge)
            valsA = small.tile([128, G], dt.float32, tag="valsA")
            nc.vector.tensor_scalar(valsA[:], iotaF[:], 1.0, None, op0=Alu.add)
            valsB = small.tile([128, G], dt.float32, tag="valsB")
            nc.vector.tensor_tensor(valsB[:], valsA[:], maskv[:], op=Alu.mult)
            valsC = small.tile([128, G], dt.float32, tag="valsC")
            nc.vector.tensor_scalar(valsC[:], valsB[:], -1.0, None, op0=Alu.add)
            nc.sync.dma_start(scr1_d, valsC[:])
            vals16 = small.tile([16, G, 8], dt.float32, tag="vals16")
            nc.sync.dma_start(vals16[:], scr1_d.rearrange("(pf pp) g -> pp g pf", pp=16))
            spout = small.tile([16, CMAX * 8], dt.float32, tag="spout")
            nf = small.tile([1, 1], dt.uint32, tag="nf")
            nc.gpsimd.sparse_gather(spout[:], vals16[:].rearrange("a b c -> a (b c)"),
                                    num_found=nf[:])

            # ---------------- slot index/validity ----------------
            nc.sync.dma_start(scr2_d, spout[:])
            idxf = small.tile([128, CMAX], dt.float32, tag="idxf")
            nc.sync.dma_start(idxf[:], scr2_d.rearrange("plo (fh j) -> (plo fh) j", j=CMAX))
            nf32 = small.tile([1, 1], dt.float32, tag="nf32")
            nc.vector.tensor_copy(nf32[:], nf[:])
            nfp = mps.tile([128, 1], dt.float32, tag="nfp")
            nc.tensor.matmul(nfp[:], ones1_32[:], nf32[:], start=True, stop=True)
            nfbs = small.tile([128, 1], dt.float32, tag="nfbs")
            nc.vector.tensor_copy(nfbs[:], nfp[:])
            mask8 = small.tile([128, CMAX], dt.float32, tag="mask8")
            nc.vector.tensor_scalar(mask8[:], slotF[:], nfbs[:, 0:1], None, op0=Alu.is_lt)
            mm8 = small.tile([128, CMAX], dt.float32, tag="mm8")
            nc.vector.tensor_scalar(mm8[:], mask8[:], 1e30, -1e30, op0=Alu.mult, op1=Alu.add)
            idx32a = small.tile([128, CMAX], dt.int32, tag="idx32a")
            nc.vector.tensor_copy(idx32a[:], idxf[:])
            idx32b = small.tile([128, CMAX], dt.int32, tag="idx32b")
            nc.vector.tensor_scalar(idx32b[:], idx32a[:], NSP - 1, None, op0=Alu.min)
            idx32 = small.tile([128, CMAX], dt.int32, tag="idx32")
            nc.vector.tensor_scalar(idx32[:], idx32b[:], 0, None, op0=Alu.max)

            # ---------------- gather + W' + den ----------------
            yhat = yhp.tile([128, CMAX, DAUG], dt.float16, tag="yhat")
            wt = wtp.tile([128, CMAX, 256], dt.float16, tag="wt")
            denp = mps.tile([1, 256], dt.float32, tag="denp")
            for j in range(CMAX):
                nc.gpsimd.indirect_dma_start(
                    out=yhat[:, j, :], out_offset=None, in_=ynat_d,
                    in_offset=bass.IndirectOffsetOnAxis(ap=idx32[:, j:j + 1], axis=0),
                    bounds_check=NSP - 1, oob_is_err=False)
                ta = tpool.tile([128, 256], dt.float32, tag="ta", name=f"ta{j}")
                nc.vector.tensor_tensor(ta[:], yhat[:, j, D:D + 256],
                                        yhat[:, j, D + 256:DAUG], op=Alu.add)
                tb = tpool.tile([128, 256], dt.float32, tag="tb", name=f"tb{j}")
                nc.vector.tensor_tensor(tb[:], ta[:], gmB32[:], op=Alu.subtract)
                nc.vector.tensor_scalar(tb[:], tb[:], mm8[:, j:j + 1], None, op0=Alu.add)
                nc.scalar.activation(wt[:, j, :], tb[:], Act.Exp)
                nc.tensor.matmul(denp[:], ones128[:], wt[:, j, :],
                                 start=(j == 0), stop=(j == CMAX - 1))

            den_s = small.tile([1, 256], dt.float32, tag="den_s")
            nc.vector.tensor_copy(den_s[:], denp[:])
            nc.sync.dma_start(den_d, den_s[:])

            # ---------------- GEMM2 (m-blocked, j-pipelined) ----------------
            for m in range(M_TILES):
                ps6 = [g2ps.tile([128, 512], dt.float32, tag="g2ps", name=f"g2_{m}_{s}")
                       for s in range(6)]
                for j in range(CMAX):
                    for s in range(6):
                        nc.tensor.matmul(
                            ps6[s][:],
                            wt[:, j, m * 128:(m + 1) * 128],
                            yhat[:, j, s * 512:(s + 1) * 512],
                            start=(j == 0), stop=(j == CMAX - 1))
                for s in range(6):
                    o = outp.tile([128, 512], dt.float32, tag="o", name=f"o{m}_{s}")
                    if m == 0:
                        nc.vector.tensor_copy(o[:], ps6[s][:])
                    else:
                        nc.scalar.activation(o[:], ps6[s][:], Act.Copy)
                    nc.sync.dma_start(num_d[m][:, s * 512:(s + 1) * 512], o[:])

    nc.compile()
    return nc


def prep_inputs(input, sigma, train_data, n_cores=N_CORES):
    """Host-side shard + pre-tile. Returns list of per-core in_maps."""
    x = np.asarray(input, dtype=np.float32).reshape(B, D)
    sig = np.asarray(sigma, dtype=np.float64)
    y = np.asarray(train_data, dtype=np.float32).reshape(N_TOTAL, D)

    is2 = (1.0 / sig ** 2).astype(np.float32)                  # [256]
    xt16 = (x * is2[:, None]).astype(np.float16)               # x~ = x*is2
    xst = np.ascontiguousarray(xt16.reshape(B, KT, 128).transpose(1, 2, 0))  # [KT,128,256]
    is2h = is2.astype(np.float16)
    is2l = (is2 - is2h.astype(np.float32)).astype(np.float16)
    is2r = np.stack([is2h, is2h, is2l])                        # [3, 256]

    chunks = chunk_list()
    in_maps = []
    for c in range(n_cores):
        ys = y[c * NS:(c + 1) * NS]
        ys16p = np.zeros((NSP, D), dtype=np.float16)
        ys16p[:NS] = ys.astype(np.float16)
        y2f = (-0.5 * np.einsum("ij,ij->i", ys.astype(np.float64),
                                ys.astype(np.float64))).astype(np.float32)
        y2h = np.full(NSP, PAD_Y2, dtype=np.float16)
        y2l = np.zeros(NSP, dtype=np.float16)
        y2h[:NS] = y2f.astype(np.float16)
        y2l[:NS] = (y2f - y2h[:NS].astype(np.float32)).astype(np.float16)
        y2t = np.stack([y2h, y2l, y2h])                        # [3, NSP]

        ynat = np.zeros((NSP, DAUG), dtype=np.float16)
        ynat[:, :D] = ys16p

        pp, jj = np.meshgrid(np.arange(128), np.arange(CMAX), indexing="ij")
        slotr = (((pp % 8) * 8 + jj) * 16 + pp // 8).astype(np.int32)
        onesel = np.zeros((2, 256), dtype=np.float32)
        onesel[0, :128] = 1.0
        onesel[1, 128:] = 1.0
        im = {"xst": xst, "is2r": is2r, "y2t": y2t, "ynat": ynat, "slotr": slotr,
              "onesel": onesel}
        for ci, (coff, csz) in enumerate(chunks):
            yt = ys16p[coff:coff + csz].T.reshape(2, KH, 128, csz)
            im[f"y_c{ci}"] = np.ascontiguousarray(yt.transpose(0, 2, 1, 3))
        in_maps.append(im)
    return in_maps


def combine(results):
    """Flash-style combine of per-core (num, den, mx) partials -> full output."""
    num = np.stack([r["num"].reshape(B, D) for r in results]).astype(np.float64)
    den = np.stack([r["den"].reshape(B) for r in results]).astype(np.float64)
    mx = np.stack([r["mx"].reshape(128, M_TILES).T.reshape(B) for r in results]).astype(np.float64)
    M = mx.max(axis=0)
    r = np.exp(mx - M[None, :])
    num_tot = (num * r[:, :, None]).sum(axis=0)
    den_tot = (den * r).sum(axis=0)
    out = (num_tot / den_tot[:, None]).astype(np.float32)
    return out.reshape(B, C, H, W_IMG)


_NC_CACHE = {}


def get_nc():
    if "nc" not in _NC_CACHE:
        _NC_CACHE["nc"] = build_nc()
    return _NC_CACHE["nc"]


def kernel(input, sigma, train_data):
    nc = get_nc()
    in_maps = prep_inputs(input, sigma, train_data)
    res = run_bass_kernel_spmd(nc, in_maps, core_ids=list(range(N_CORES)))
    return combine(res.results)


# revision 34
# speedup vs baseline: 1.3875x; 1.1177x over previous
"""Karras optimal denoiser on 8 Trainium2 NeuronCores — sparse-GEMM2 version.

Math: out_b = sum_i w_bi y_i / sum_i w_bi,  w_bi = exp((x_b.y_i - 0.5||y_i||^2)/sigma_b^2).
Softmax over N=50000 is extremely peaked (retrieval-knn): per core only ~700 of
6250 columns have weight mass >= tau=exp(-14) relative to the local row max.

Per-core plan (train_data sharded over N, flash-style host combine):
  Phase 1 (GEMM1, transposed): LamT[i, b] = (x_b*is2_b . y_i) + (-0.5||y_i||^2)*is2_b
    computed 128-column-group at a time: stationary = y^T k-tiles, moving = x~^T.
    The y2*is2 term enters as a K=3 matmul (hi/lo fp16 split of both factors).
    Per group: PSUM -> hi16 (fp16) + lo16 (fp16 residual, exact to ~2^-12),
    both DMA'd into spare columns of the ynat DRAM tensor (rows = train points);
    hi16 also kept in SBUF. Row maxes via 2 PE transposes + DVE reduce_max.
  Selection: keepscore_i = max_b (hi - gm~_b); columns with keepscore >= ln(tau)
    are compacted to a rank list by gpsimd sparse_gather ([16,F] wrapped layout;
    DRAM-bounce rearranges to/from it). Slots are rank-ordered => perfectly
    partition-balanced. Slot validity from num_found (HW tail is garbage).
  Gather: per slot-block j, one indirect DMA fetches 128 rows of
    ynat = [y row fp16 (3072) | hi (256) | lo (256)] -> 7KB rows at ~line rate.
  W'^T_j = exp(hi + lo - gm~) (invalid slots killed via -1e30 bias); den = ones^T W'.
  GEMM2: num = W'^T.T @ yhat over CMAX j-blocks, m-blocked (6 PSUM banks),
    pipelined against the gathers.
Outputs per core: num [2,128,3072], den [1,256], mx [128,2] (=gm~, scaled-logit
frame); host does the flash combine in that frame.
"""

import numpy as np
from contextlib import ExitStack

import concourse.bass as bass
import concourse.tile as tile
import concourse.mybir as mybir
from concourse import bacc
from concourse.bass_utils import run_bass_kernel_spmd
from concourse.masks import make_identity

dt = mybir.dt
Alu = mybir.AluOpType
Act = mybir.ActivationFunctionType

B, C, H, W_IMG = 256, 3, 32, 32
D = C * H * W_IMG            # 3072
N_TOTAL = 50000
N_CORES = 8
NS = N_TOTAL // N_CORES      # 6250 per core
NSP = 6272                   # padded to 49*128
KT = D // 128                # 24 contraction k-tiles
KH = KT // 2                 # 12 per stream half
G = NSP // 128               # 49 column groups
M_TILES = 2                  # 256 query rows = 2 partition tiles
CMAX = 8                     # survivor slot blocks (cap = 128*CMAX = 1024)
LNTAU = -14.0                # keep threshold on log-weights
DAUG = D + 512               # ynat row: y | hi | lo
PAD_Y2 = -15000.0            # pad-row -0.5||y||^2 sentinel (never selected; *4 stays in fp16)


def chunk_list(nsp=NSP):
    """128-multiple chunks, ramped so the first y DMAs land fast."""
    sizes = [128, 256, 384]
    out, off = [], 0
    for s in sizes:
        out.append((off, s))
        off += s
    while off < nsp:
        out.append((off, min(512, nsp - off)))
        off += 512
    return out


def build_nc():
    chunks = chunk_list()
    nc = bacc.Bacc("TRN2", target_bir_lowering=False, debug=False)

    # --- DRAM I/O ---
    y_d = [nc.dram_tensor(f"y_c{ci}", (2, 128, KH, csz), dt.float16, kind="ExternalInput").ap()
           for ci, (_, csz) in enumerate(chunks)]
    y2t_d = nc.dram_tensor("y2t", (3, NSP), dt.float16, kind="ExternalInput").ap()
    is2r_d = nc.dram_tensor("is2r", (3, 256), dt.float16, kind="ExternalInput").ap()
    xst_d = nc.dram_tensor("xst", (KT, 128, 256), dt.float16, kind="ExternalInput").ap()
    ynat_d = nc.dram_tensor("ynat", (NSP, DAUG), dt.float16, kind="ExternalInput").ap()

    num_d = nc.dram_tensor("num", (M_TILES, 128, D), dt.float32, kind="ExternalOutput").ap()
    den_d = nc.dram_tensor("den", (1, 256), dt.float32, kind="ExternalOutput").ap()
    slotr_d = nc.dram_tensor("slotr", (128, CMAX), dt.int32, kind="ExternalInput").ap()
    onesel_d = nc.dram_tensor("onesel", (2, 256), dt.float32, kind="ExternalInput").ap()
    mx_d = nc.dram_tensor("mx", (1, 128, M_TILES), dt.float32, kind="ExternalOutput").ap()
    scr1_d = nc.dram_tensor("scr1", (128, G), dt.float32, kind="ExternalOutput").ap()
    scr2_d = nc.dram_tensor("scr2", (16, CMAX * 8), dt.float32, kind="ExternalOutput").ap()

    with tile.TileContext(nc) as tc:
        with ExitStack() as ctx:
            small = ctx.enter_context(tc.tile_pool(name="small", bufs=1))
            xpool = ctx.enter_context(tc.tile_pool(name="x", bufs=1))
            ypool = ctx.enter_context(tc.tile_pool(name="ystream", bufs=4))
            hipool = ctx.enter_context(tc.tile_pool(name="hi", bufs=1))
            spool = ctx.enter_context(tc.tile_pool(name="ssel", bufs=2))
            tpool = ctx.enter_context(tc.tile_pool(name="texp", bufs=2))
            wtp = ctx.enter_context(tc.tile_pool(name="wt", bufs=1))
            yhp = ctx.enter_context(tc.tile_pool(name="yhat", bufs=1))
            outp = ctx.enter_context(tc.tile_pool(name="odrain", bufs=3))
            ps_ctx = ExitStack()
            g1ps = ps_ctx.enter_context(tc.tile_pool(name="g1ps", bufs=2, space="PSUM"))
            tps = ps_ctx.enter_context(tc.tile_pool(name="tps", bufs=2, space="PSUM"))

            # ---- constants / small state ----
            ident = small.tile([128, 128], dt.float16, tag="ident")
            make_identity(nc, ident[:])
            ident32 = small.tile([128, 128], dt.float32, tag="ident32")
            nc.vector.tensor_copy(ident32[:], ident[:])
            ones128 = small.tile([128, 1], dt.float16, tag="ones128")
            nc.vector.memset(ones128[:], 1.0)
            ones1_32 = small.tile([1, 128], dt.float32, tag="ones1_32")
            nc.vector.memset(ones1_32[:], 1.0)
            rm = small.tile([128, M_TILES], dt.float32, tag="rm")
            colmax = small.tile([128, M_TILES * G], dt.float32, tag="colmax")
            iota_i = small.tile([128, G], dt.int32, tag="iota_i")
            nc.gpsimd.iota(iota_i[:], pattern=[[128, G]], base=0, channel_multiplier=1)
            iotaF = small.tile([128, G], dt.float32, tag="iotaF")
            nc.vector.tensor_copy(iotaF[:], iota_i[:])
            slot_i = small.tile([128, CMAX], dt.int32, tag="slot_i")
            nc.sync.dma_start(slot_i[:], slotr_d)
            slotF = small.tile([128, CMAX], dt.float32, tag="slotF")
            nc.vector.tensor_copy(slotF[:], slot_i[:])
            onesel = small.tile([2, 256], dt.float32, tag="onesel")
            nc.sync.dma_start(onesel[:], onesel_d)

            is2r_t = small.tile([3, 256], dt.float16, tag="is2r")
            nc.sync.dma_start(is2r_t[:], is2r_d)
            y2t_t = small.tile([3, NSP], dt.float16, tag="y2t")
            nc.sync.dma_start(y2t_t[:], y2t_d)
            xst_t = xpool.tile([128, KT, 256], dt.float16, tag="xst")
            nc.sync.dma_start(xst_t[:], xst_d.rearrange("k p b -> p k b"))

            # hilo[:, g*512 : g*512+256] = hi (fp16 of LamT), [+256:+512] = lo residual
            hilo = hipool.tile([128, G * 512], dt.float16, tag="hilo")

            def emit_transposes(g):
                """PE transposes of group g's hi (for row maxes) — emitted a few
                groups late so PE never waits on the ACT hi-copy."""
                for m in range(M_TILES):
                    tp = tps.tile([128, 128], dt.float16, tag="tp", name=f"tp{g}_{m}")
                    nc.tensor.matmul(tp[:], hilo[:, g * 512 + m * 128:g * 512 + (m + 1) * 128],
                                     ident[:], is_transpose=True, start=True, stop=True)
                    nc.vector.reduce_max(colmax[:, m * G + g:m * G + g + 1], tp[:],
                                         mybir.AxisListType.X)

            # ---------------- Phase 1: transposed GEMM1 ----------------
            TDELAY = 2
            for ci, (coff, csz) in enumerate(chunks):
                yh = [ypool.tile([128, KH, csz], dt.float16, tag="y", name=f"y{ci}_{h}")
                      for h in range(2)]
                for h in range(2):
                    nc.sync.dma_start(yh[h][:], y_d[ci][h])
                for gi in range(csz // 128):
                    g = coff // 128 + gi
                    ps = g1ps.tile([128, 256], dt.float32, tag="g1ps", name=f"ps{g}")
                    nc.tensor.matmul(ps[:], y2t_t[:, g * 128:(g + 1) * 128],
                                     is2r_t[:], start=True, stop=False)
                    for k in range(KT):
                        nc.tensor.matmul(
                            ps[:],
                            yh[k // KH][:, k % KH, gi * 128:(gi + 1) * 128],
                            xst_t[:, k, :],
                            start=False, stop=(k == KT - 1))
                    hs = hilo[:, g * 512:g * 512 + 256]
                    nc.scalar.activation(hs, ps[:], Act.Copy)
                    lo = hilo[:, g * 512 + 256:g * 512 + 512]
                    nc.vector.tensor_tensor(lo, ps[:], hs, op=Alu.subtract)
                    nc.scalar.dma_start(ynat_d[g * 128:(g + 1) * 128, D:DAUG],
                                        hilo[:, g * 512:(g + 1) * 512])
                    if g >= TDELAY:
                        emit_transposes(g - TDELAY)
            for g in range(G - TDELAY, G):
                emit_transposes(g)
            for m in range(M_TILES):
                nc.vector.reduce_max(rm[:, m:m + 1], colmax[:, m * G:(m + 1) * G],
                                     mybir.AxisListType.X)

            # ---------------- gm broadcast (PE transpose, no DRAM bounce) ----------------
            nc.sync.dma_start(mx_d[0], rm[:])
            tpr = tps.tile([2, 128], dt.float32, tag="tpr")
            nc.tensor.matmul(tpr[:], rm[:], ident32[:], is_transpose=True, start=True, stop=True)
            gsm2 = small.tile([2, 128], dt.float32, tag="gsm2")
            nc.vector.tensor_copy(gsm2[:], tpr[:])
            bps = g1ps.tile([128, 256], dt.float32, tag="g1ps", name="bps")
            for m in range(M_TILES):
                nc.tensor.matmul(bps[:, m * 128:(m + 1) * 128], onesel[:, m * 128:(m + 1) * 128],
                                 gsm2[:], start=True, stop=True)
            gmB32 = small.tile([128, 256], dt.float32, tag="gmB32")
            nc.vector.tensor_copy(gmB32[:], bps[:])
            gmB16 = small.tile([128, 256], dt.float16, tag="gmB16")
            nc.scalar.activation(gmB16[:], bps[:], Act.Copy)

            ps_ctx.close()   # free phase-1 PSUM banks
            g2ps = ctx.enter_context(tc.tile_pool(name="g2ps", bufs=6, space="PSUM"))
            mps = ctx.enter_context(tc.tile_pool(name="mps", bufs=1, space="PSUM"))

            # ---------------- selection (batched 4 groups per DVE op) ----------------
            gmRep = small.tile([128, 4, 256], dt.float16, tag="gmRep")
            for q in range(4):
                nc.vector.tensor_copy(gmRep[:, q, :], gmB16[:])
            keep = small.tile([128, G], dt.float32, tag="keep")
            g0 = 0
            while g0 < G:
                gb = min(4, G - g0)
                s = spool.tile([128, 4, 256], dt.float16, tag="s", name=f"s{g0}")
                his = hilo[:, g0 * 512:(g0 + gb) * 512].rearrange(
                    "p (g c) -> p g c", c=512)[:, :, 0:256]
                nc.vector.tensor_tensor(s[:, :gb, :], his, gmRep[:, :gb, :],
                                        op=Alu.subtract)
                nc.vector.reduce_max(keep[:, g0:g0 + gb], s[:, :gb, :],
                                     mybir.AxisListType.X)
                g0 += gb

            maskv = small.tile([128, G], dt.float32, tag="maskv")
            nc.vector.tensor_scalar(maskv[:], keep[:], LNTAU, None, op0=Alu.is_ge)
            valsA = small.tile([128, G], dt.float32, tag="valsA")
            nc.vector.tensor_scalar(valsA[:], iotaF[:], 1.0, None, op0=Alu.add)
            valsB = small.tile([128, G], dt.float32, tag="valsB")
            nc.vector.tensor_tensor(valsB[:], valsA[:], maskv[:], op=Alu.mult)
            valsC = small.tile([128, G], dt.float32, tag="valsC")
            nc.vector.tensor_scalar(valsC[:], valsB[:], -1.0, None, op0=Alu.add)
            nc.sync.dma_start(scr1_d, valsC[:])
            vals16 = small.tile([16, G, 8], dt.float32, tag="vals16")
            nc.sync.dma_start(vals16[:], scr1_d.rearrange("(pf pp) g -> pp g pf", pp=16))
            spout = small.tile([16, CMAX * 8], dt.float32, tag="spout")
            nf = small.tile([1, 1], dt.uint32, tag="nf")
            nc.gpsimd.sparse_gather(spout[:], vals16[:].rearrange("a b c -> a (b c)"),
                                    num_found=nf[:])

            # ---------------- slot index/validity ----------------
            nc.sync.dma_start(scr2_d, spout[:])
            idxf = small.tile([128, CMAX], dt.float32, tag="idxf")
            nc.sync.dma_start(idxf[:], scr2_d.rearrange("plo (fh j) -> (plo fh) j", j=CMAX))
            nf32 = small.tile([1, 1], dt.float32, tag="nf32")
            nc.vector.tensor_copy(nf32[:], nf[:])
            nfp = mps.tile([128, 1], dt.float32, tag="nfp")
            nc.tensor.matmul(nfp[:], ones1_32[:], nf32[:], start=True, stop=True)
            nfbs = small.tile([128, 1], dt.float32, tag="nfbs")
            nc.vector.tensor_copy(nfbs[:], nfp[:])
            mask8 = small.tile([128, CMAX], dt.float32, tag="mask8")
            nc.vector.tensor_scalar(mask8[:], slotF[:], nfbs[:, 0:1], None, op0=Alu.is_lt)
            mm8 = small.tile([128, CMAX], dt.float32, tag="mm8")
            nc.vector.tensor_scalar(mm8[:], mask8[:], 1e30, -1e30, op0=Alu.mult, op1=Alu.add)
            idx32a = small.tile([128, CMAX], dt.int32, tag="idx32a")
            nc.vector.tensor_copy(idx32a[:], idxf[:])
            idx32b = small.tile([128, CMAX], dt.int32, tag="idx32b")
            nc.vector.tensor_scalar(idx32b[:], idx32a[:], NSP - 1, None, op0=Alu.min)
            idx32 = small.tile([128, CMAX], dt.int32, tag="idx32")
            nc.vector.tensor_scalar(idx32[:], idx32b[:], 0, None, op0=Alu.max)

            # ---------------- gather + W' + den ----------------
            yhat = yhp.tile([128, CMAX, DAUG], dt.float16, tag="yhat")
            wt = wtp.tile([128, CMAX, 256], dt.float16, tag="wt")
            denp = mps.tile([1, 256], dt.float32, tag="denp")
            for j in range(CMAX):
                nc.gpsimd.indirect_dma_start(
                    out=yhat[:, j, :], out_offset=None, in_=ynat_d,
                    in_offset=bass.IndirectOffsetOnAxis(ap=idx32[:, j:j + 1], axis=0),
                    bounds_check=NSP - 1, oob_is_err=False)
                ta = tpool.tile([128, 256], dt.float32, tag="ta", name=f"ta{j}")
                nc.vector.tensor_tensor(ta[:], yhat[:, j, D:D + 256],
                                        yhat[:, j, D + 256:DAUG], op=Alu.add)
                tb = tpool.tile([128, 256], dt.float32, tag="tb", name=f"tb{j}")
                nc.vector.tensor_tensor(tb[:], ta[:], gmB32[:], op=Alu.subtract)
                nc.vector.tensor_scalar(tb[:], tb[:], mm8[:, j:j + 1], None, op0=Alu.add)
                nc.scalar.activation(wt[:, j, :], tb[:], Act.Exp)
                nc.tensor.matmul(denp[:], ones128[:], wt[:, j, :],
                                 start=(j == 0), stop=(j == CMAX - 1))

            den_s = small.tile([1, 256], dt.float32, tag="den_s")
            nc.vector.tensor_copy(den_s[:], denp[:])
            nc.sync.dma_start(den_d, den_s[:])

            # ---------------- GEMM2 (m-blocked, j-pipelined) ----------------
            for m in range(M_TILES):
                ps6 = [g2ps.tile([128, 512], dt.float32, tag="g2ps", name=f"g2_{m}_{s}")
                       for s in range(6)]
                for j in range(CMAX):
                    for s in range(6):
                        nc.tensor.matmul(
                            ps6[s][:],
                            wt[:, j, m * 128:(m + 1) * 128],
                            yhat[:, j, s * 512:(s + 1) * 512],
                            start=(j == 0), stop=(j == CMAX - 1))
                for s in range(6):
                    o = outp.tile([128, 512], dt.float32, tag="o", name=f"o{m}_{s}")
                    if m == 0:
                        nc.vector.tensor_copy(o[:], ps6[s][:])
                    else:
                        nc.scalar.activation(o[:], ps6[s][:], Act.Copy)
                    nc.sync.dma_start(num_d[m][:, s * 512:(s + 1) * 512], o[:])

    nc.compile()
    return nc


def prep_inputs(input, sigma, train_data, n_cores=N_CORES):
    """Host-side shard + pre-tile. Returns list of per-core in_maps."""
    x = np.asarray(input, dtype=np.float32).reshape(B, D)
    sig = np.asarray(sigma, dtype=np.float64)
    y = np.asarray(train_data, dtype=np.float32).reshape(N_TOTAL, D)

    is2 = (1.0 / sig ** 2).astype(np.float32)                  # [256]
    xt16 = (x * is2[:, None]).astype(np.float16)               # x~ = x*is2
    xst = np.ascontiguousarray(xt16.reshape(B, KT, 128).transpose(1, 2, 0))  # [KT,128,256]
    is2h = is2.astype(np.float16)
    is2l = (is2 - is2h.astype(np.float32)).astype(np.float16)
    is2r = np.stack([is2h, is2h, is2l])                        # [3, 256]

    chunks = chunk_list()
    in_maps = []
    for c in range(n_cores):
        ys = y[c * NS:(c + 1) * NS]
        ys16p = np.zeros((NSP, D), dtype=np.float16)
        ys16p[:NS] = ys.astype(np.float16)
        y2f = (-0.5 * np.einsum("ij,ij->i", ys.astype(np.float64),
                                ys.astype(np.float64))).astype(np.float32)
        y2h = np.full(NSP, PAD_Y2, dtype=np.float16)
        y2l = np.zeros(NSP, dtype=np.float16)
        y2h[:NS] = y2f.astype(np.float16)
        y2l[:NS] = (y2f - y2h[:NS].astype(np.float32)).astype(np.float16)
        y2t = np.stack([y2h, y2l, y2h])                        # [3, NSP]

        ynat = np.zeros((NSP, DAUG), dtype=np.float16)
        ynat[:, :D] = ys16p

        pp, jj = np.meshgrid(np.arange(128), np.arange(CMAX), indexing="ij")
        slotr = (((pp % 8) * 8 + jj) * 16 + pp // 8).astype(np.int32)
        onesel = np.zeros((2, 256), dtype=np.float32)
        onesel[0, :128] = 1.0
        onesel[1, 128:] = 1.0
        im = {"xst": xst, "is2r": is2r, "y2t": y2t, "ynat": ynat, "slotr": slotr,
              "onesel": onesel}
        for ci, (coff, csz) in enumerate(chunks):
            yt = ys16p[coff:coff + csz].T.reshape(2, KH, 128, csz)
            im[f"y_c{ci}"] = np.ascontiguousarray(yt.transpose(0, 2, 1, 3))
        in_maps.append(im)
    return in_maps


def combine(results):
    """Flash-style combine of per-core (num, den, mx) partials -> full output."""
    num = np.stack([r["num"].reshape(B, D) for r in results]).astype(np.float64)
    den = np.stack([r["den"].reshape(B) for r in results]).astype(np.float64)
    mx = np.stack([r["mx"].reshape(128, M_TILES).T.reshape(B) for r in results]).astype(np.float64)
    M = mx.max(axis=0)
    r = np.exp(mx - M[None, :])
    num_tot = (num * r[:, :, None]).sum(axis=0)
    den_tot = (den * r).sum(axis=0)
    out = (num_tot / den_tot[:, None]).astype(np.float32)
    return out.reshape(B, C, H, W_IMG)


_NC_CACHE = {}


def get_nc():
    if "nc" not in _NC_CACHE:
        _NC_CACHE["nc"] = build_nc()
    return _NC_CACHE["nc"]


def kernel(input, sigma, train_data):
    nc = get_nc()
    in_maps = prep_inputs(input, sigma, train_data)
    res = run_bass_kernel_spmd(nc, in_maps, core_ids=list(range(N_CORES)))
    return combine(res.results)


# revision 37
# speedup vs baseline: 1.4080x; 1.0148x over previous
"""Karras optimal denoiser on 8 Trainium2 NeuronCores — sparse-GEMM2 version.

Math: out_b = sum_i w_bi y_i / sum_i w_bi,  w_bi = exp((x_b.y_i - 0.5||y_i||^2)/sigma_b^2).
Softmax over N=50000 is extremely peaked (retrieval-knn): per core only ~700 of
6250 columns have weight mass >= tau=exp(-14) relative to the local row max.

Per-core plan (train_data sharded over N, flash-style host combine):
  Phase 1 (GEMM1, transposed): LamT[i, b] = (x_b*is2_b . y_i) + (-0.5||y_i||^2)*is2_b
    computed 128-column-group at a time: stationary = y^T k-tiles, moving = x~^T.
    The y2*is2 term enters as a K=3 matmul (hi/lo fp16 split of both factors).
    Per group: PSUM -> hi16 (fp16) + lo16 (fp16 residual, exact to ~2^-12),
    both DMA'd into spare columns of the ynat DRAM tensor (rows = train points);
    hi16 also kept in SBUF. Row maxes via 2 PE transposes + DVE reduce_max.
  Selection: keepscore_i = max_b (hi - gm~_b); columns with keepscore >= ln(tau)
    are compacted to a rank list by gpsimd sparse_gather ([16,F] wrapped layout;
    DRAM-bounce rearranges to/from it). Slots are rank-ordered => perfectly
    partition-balanced. Slot validity from num_found (HW tail is garbage).
  Gather: per slot-block j, one indirect DMA fetches 128 rows of
    ynat = [y row fp16 (3072) | hi (256) | lo (256)] -> 7KB rows at ~line rate.
  W'^T_j = exp(hi + lo - gm~) (invalid slots killed via -1e30 bias); den = ones^T W'.
  GEMM2: num = W'^T.T @ yhat over CMAX j-blocks, m-blocked (6 PSUM banks),
    pipelined against the gathers.
Outputs per core: num [2,128,3072], den [1,256], mx [128,2] (=gm~, scaled-logit
frame); host does the flash combine in that frame.
"""

import numpy as np
from contextlib import ExitStack

import concourse.bass as bass
import concourse.tile as tile
import concourse.mybir as mybir
from concourse import bacc, bass_isa
from concourse.bass_utils import run_bass_kernel_spmd
from concourse.masks import make_identity

dt = mybir.dt
Alu = mybir.AluOpType
Act = mybir.ActivationFunctionType

B, C, H, W_IMG = 256, 3, 32, 32
D = C * H * W_IMG            # 3072
N_TOTAL = 50000
N_CORES = 8
NS = N_TOTAL // N_CORES      # 6250 per core
NSP = 6272                   # padded to 49*128
KT = D // 128                # 24 contraction k-tiles
KH = KT // 2                 # 12 per stream half
G = NSP // 128               # 49 column groups
M_TILES = 2                  # 256 query rows = 2 partition tiles
CMAX = 8                     # survivor slot blocks (cap = 128*CMAX = 1024)
LNTAU = -14.0                # keep threshold on log-weights
DAUG = D + 512               # ynat row: y | hi | lo
PAD_Y2 = -15000.0            # pad-row -0.5||y||^2 sentinel (never selected; *4 stays in fp16)


def chunk_list(nsp=NSP):
    """128-multiple chunks, ramped so the first y DMAs land fast."""
    sizes = [128, 256, 384]
    out, off = [], 0
    for s in sizes:
        out.append((off, s))
        off += s
    while off < nsp:
        out.append((off, min(512, nsp - off)))
        off += 512
    return out


def build_nc():
    chunks = chunk_list()
    nc = bacc.Bacc("TRN2", target_bir_lowering=False, debug=False)

    # --- DRAM I/O ---
    y_d = [nc.dram_tensor(f"y_c{ci}", (2, 128, KH, csz), dt.float16, kind="ExternalInput").ap()
           for ci, (_, csz) in enumerate(chunks)]
    y2t_d = nc.dram_tensor("y2t", (3, NSP), dt.float16, kind="ExternalInput").ap()
    is2r_d = nc.dram_tensor("is2r", (3, 256), dt.float16, kind="ExternalInput").ap()
    xst_d = nc.dram_tensor("xst", (KT, 128, 256), dt.float16, kind="ExternalInput").ap()
    ynat_d = nc.dram_tensor("ynat", (NSP, DAUG), dt.float16, kind="ExternalInput").ap()

    num_d = nc.dram_tensor("num", (M_TILES, 128, D), dt.float32, kind="ExternalOutput").ap()
    den_d = nc.dram_tensor("den", (1, 256), dt.float32, kind="ExternalOutput").ap()
    slotr_d = nc.dram_tensor("slotr", (128, CMAX), dt.int32, kind="ExternalInput").ap()
    mx_d = nc.dram_tensor("mx", (1, 256), dt.float32, kind="ExternalOutput").ap()
    scr1_d = nc.dram_tensor("scr1", (128, G), dt.float32, kind="ExternalOutput").ap()
    scr2_d = nc.dram_tensor("scr2", (16, CMAX * 8), dt.float32, kind="ExternalOutput").ap()

    with tile.TileContext(nc) as tc:
        with ExitStack() as ctx:
            small = ctx.enter_context(tc.tile_pool(name="small", bufs=1))
            xpool = ctx.enter_context(tc.tile_pool(name="x", bufs=1))
            ypool = ctx.enter_context(tc.tile_pool(name="ystream", bufs=4))
            hipool = ctx.enter_context(tc.tile_pool(name="hi", bufs=1))
            spool = ctx.enter_context(tc.tile_pool(name="ssel", bufs=2))
            tpool = ctx.enter_context(tc.tile_pool(name="texp", bufs=2))
            wtp = ctx.enter_context(tc.tile_pool(name="wt", bufs=1))
            yhp = ctx.enter_context(tc.tile_pool(name="yhat", bufs=1))
            outp = ctx.enter_context(tc.tile_pool(name="odrain", bufs=3))
            pmaxp = ctx.enter_context(tc.tile_pool(name="pmax", bufs=2))
            ps_ctx = ExitStack()
            g1ps = ps_ctx.enter_context(tc.tile_pool(name="g1ps", bufs=2, space="PSUM"))

            # ---- constants / small state ----
            ones128 = small.tile([128, 1], dt.float16, tag="ones128")
            nc.vector.memset(ones128[:], 1.0)
            ones1_32 = small.tile([1, 128], dt.float32, tag="ones1_32")
            nc.vector.memset(ones1_32[:], 1.0)
            gmB32 = small.tile([128, 256], dt.float32, tag="gmB32")
            nc.vector.memset(gmB32[:], -1e30)
            iota_i = small.tile([128, G], dt.int32, tag="iota_i")
            nc.gpsimd.iota(iota_i[:], pattern=[[128, G]], base=0, channel_multiplier=1)
            iotaF = small.tile([128, G], dt.float32, tag="iotaF")
            nc.vector.tensor_copy(iotaF[:], iota_i[:])
            slot_i = small.tile([128, CMAX], dt.int32, tag="slot_i")
            nc.sync.dma_start(slot_i[:], slotr_d)
            slotF = small.tile([128, CMAX], dt.float32, tag="slotF")
            nc.vector.tensor_copy(slotF[:], slot_i[:])

            is2r_t = small.tile([3, 256], dt.float16, tag="is2r")
            nc.sync.dma_start(is2r_t[:], is2r_d)
            y2t_t = small.tile([3, NSP], dt.float16, tag="y2t")
            nc.sync.dma_start(y2t_t[:], y2t_d)
            xst_t = xpool.tile([128, KT, 256], dt.float16, tag="xst")
            nc.sync.dma_start(xst_t[:], xst_d.rearrange("k p b -> p k b"))

            # hilo[:, g*512 : g*512+256] = hi (fp16 of LamT), [+256:+512] = lo residual
            hilo = hipool.tile([128, G * 512], dt.float16, tag="hilo")

            # ---------------- Phase 1: transposed GEMM1 ----------------
            for ci, (coff, csz) in enumerate(chunks):
                yh = [ypool.tile([128, KH, csz], dt.float16, tag="y", name=f"y{ci}_{h}")
                      for h in range(2)]
                for h in range(2):
                    nc.sync.dma_start(yh[h][:], y_d[ci][h])
                for gi in range(csz // 128):
                    g = coff // 128 + gi
                    ps = g1ps.tile([128, 256], dt.float32, tag="g1ps", name=f"ps{g}")
                    nc.tensor.matmul(ps[:], y2t_t[:, g * 128:(g + 1) * 128],
                                     is2r_t[:], start=True, stop=False)
                    for k in range(KT):
                        nc.tensor.matmul(
                            ps[:],
                            yh[k // KH][:, k % KH, gi * 128:(gi + 1) * 128],
                            xst_t[:, k, :],
                            start=False, stop=(k == KT - 1))
                    hs = hilo[:, g * 512:g * 512 + 256]
                    nc.scalar.activation(hs, ps[:], Act.Copy)
                    lo = hilo[:, g * 512 + 256:g * 512 + 512]
                    nc.vector.tensor_tensor(lo, ps[:], hs, op=Alu.subtract)
                    nc.scalar.dma_start(ynat_d[g * 128:(g + 1) * 128, D:DAUG],
                                        hilo[:, g * 512:(g + 1) * 512])
                    pm = pmaxp.tile([128, 256], dt.float32, tag="pm", name=f"pm{g}")
                    nc.gpsimd.partition_all_reduce(pm[:], hs, channels=128,
                                                   reduce_op=bass_isa.ReduceOp.max)
                    nc.vector.tensor_tensor(gmB32[:], gmB32[:], pm[:], op=Alu.max)

            # ---------------- gm finalize ----------------
            nc.sync.dma_start(mx_d, gmB32[0:1, :])
            gmB16 = small.tile([128, 256], dt.float16, tag="gmB16")
            nc.scalar.activation(gmB16[:], gmB32[:], Act.Copy)

            ps_ctx.close()   # free phase-1 PSUM banks
            g2ps = ctx.enter_context(tc.tile_pool(name="g2ps", bufs=6, space="PSUM"))
            mps = ctx.enter_context(tc.tile_pool(name="mps", bufs=1, space="PSUM"))

            # ---------------- selection (batched 4 groups per DVE op) ----------------
            gmRep = small.tile([128, 4, 256], dt.float16, tag="gmRep")
            for q in range(4):
                nc.vector.tensor_copy(gmRep[:, q, :], gmB16[:])
            keep = small.tile([128, G], dt.float32, tag="keep")
            g0 = 0
            while g0 < G:
                gb = min(4, G - g0)
                s = spool.tile([128, 4, 256], dt.float16, tag="s", name=f"s{g0}")
                his = hilo[:, g0 * 512:(g0 + gb) * 512].rearrange(
                    "p (g c) -> p g c", c=512)[:, :, 0:256]
                nc.vector.tensor_tensor(s[:, :gb, :], his, gmRep[:, :gb, :],
                                        op=Alu.subtract)
                nc.vector.reduce_max(keep[:, g0:g0 + gb], s[:, :gb, :],
                                     mybir.AxisListType.X)
                g0 += gb

            maskv = small.tile([128, G], dt.float32, tag="maskv")
            nc.vector.tensor_scalar(maskv[:], keep[:], LNTAU, None, op0=Alu.is_ge)
            valsA = small.tile([128, G], dt.float32, tag="valsA")
            nc.vector.tensor_scalar(valsA[:], iotaF[:], 1.0, None, op0=Alu.add)
            valsB = small.tile([128, G], dt.float32, tag="valsB")
            nc.vector.tensor_tensor(valsB[:], valsA[:], maskv[:], op=Alu.mult)
            valsC = small.tile([128, G], dt.float32, tag="valsC")
            nc.vector.tensor_scalar(valsC[:], valsB[:], -1.0, None, op0=Alu.add)
            nc.sync.dma_start(scr1_d, valsC[:])
            vals16 = small.tile([16, 8, G], dt.float32, tag="vals16")
            nc.sync.dma_start(vals16[:],
                              scr1_d.rearrange("(pf pp) g -> pp pf g", pp=16))
            spout = small.tile([16, CMAX * 8], dt.float32, tag="spout")
            nf = small.tile([1, 1], dt.uint32, tag="nf")
            nc.gpsimd.sparse_gather(spout[:], vals16[:].rearrange("a b c -> a (b c)"),
                                    num_found=nf[:])

            # ---------------- slot index/validity ----------------
            nc.sync.dma_start(scr2_d, spout[:])
            idxf = small.tile([128, CMAX], dt.float32, tag="idxf")
            nc.sync.dma_start(idxf[:], scr2_d.rearrange("plo (fh j) -> (plo fh) j", j=CMAX))
            nf32 = small.tile([1, 1], dt.float32, tag="nf32")
            nc.vector.tensor_copy(nf32[:], nf[:])
            nfp = mps.tile([128, 1], dt.float32, tag="nfp")
            nc.tensor.matmul(nfp[:], ones1_32[:], nf32[:], start=True, stop=True)
            nfbs = small.tile([128, 1], dt.float32, tag="nfbs")
            nc.vector.tensor_copy(nfbs[:], nfp[:])
            mask8 = small.tile([128, CMAX], dt.float32, tag="mask8")
            nc.vector.tensor_scalar(mask8[:], slotF[:], nfbs[:, 0:1], None, op0=Alu.is_lt)
            mm8 = small.tile([128, CMAX], dt.float32, tag="mm8")
            nc.vector.tensor_scalar(mm8[:], mask8[:], 1e30, -1e30, op0=Alu.mult, op1=Alu.add)
            idx32a = small.tile([128, CMAX], dt.int32, tag="idx32a")
            nc.vector.tensor_copy(idx32a[:], idxf[:])
            idx32b = small.tile([128, CMAX], dt.int32, tag="idx32b")
            nc.vector.tensor_scalar(idx32b[:], idx32a[:], NSP - 1, None, op0=Alu.min)
            idx32 = small.tile([128, CMAX], dt.int32, tag="idx32")
            nc.vector.tensor_scalar(idx32[:], idx32b[:], 0, None, op0=Alu.max)

            # ---------------- gather + W' + den ----------------
            yhat = yhp.tile([128, CMAX, DAUG], dt.float16, tag="yhat")
            wt = wtp.tile([128, CMAX, 256], dt.float16, tag="wt")
            denp = mps.tile([1, 256], dt.float32, tag="denp")
            for j in range(CMAX):
                nc.gpsimd.indirect_dma_start(
                    out=yhat[:, j, :], out_offset=None, in_=ynat_d,
                    in_offset=bass.IndirectOffsetOnAxis(ap=idx32[:, j:j + 1], axis=0),
                    bounds_check=NSP - 1, oob_is_err=False)
                ta = tpool.tile([128, 256], dt.float32, tag="ta", name=f"ta{j}")
                nc.vector.tensor_tensor(ta[:], yhat[:, j, D:D + 256],
                                        yhat[:, j, D + 256:DAUG], op=Alu.add)
                tb = tpool.tile([128, 256], dt.float32, tag="tb", name=f"tb{j}")
                nc.vector.tensor_tensor(tb[:], ta[:], gmB32[:], op=Alu.subtract)
                nc.vector.tensor_scalar(tb[:], tb[:], mm8[:, j:j + 1], None, op0=Alu.add)
                nc.scalar.activation(wt[:, j, :], tb[:], Act.Exp)
                nc.tensor.matmul(denp[:], ones128[:], wt[:, j, :],
                                 start=(j == 0), stop=(j == CMAX - 1))

            den_s = small.tile([1, 256], dt.float32, tag="den_s")
            nc.vector.tensor_copy(den_s[:], denp[:])
            nc.sync.dma_start(den_d, den_s[:])

            # ---------------- GEMM2 (m-blocked, j-pipelined) ----------------
            for m in range(M_TILES):
                ps6 = [g2ps.tile([128, 512], dt.float32, tag="g2ps", name=f"g2_{m}_{s}")
                       for s in range(6)]
                for j in range(CMAX):
                    for s in range(6):
                        nc.tensor.matmul(
                            ps6[s][:],
                            wt[:, j, m * 128:(m + 1) * 128],
                            yhat[:, j, s * 512:(s + 1) * 512],
                            start=(j == 0), stop=(j == CMAX - 1))
                for s in range(6):
                    o = outp.tile([128, 512], dt.float32, tag="o", name=f"o{m}_{s}")
                    if m == 0:
                        nc.vector.tensor_copy(o[:], ps6[s][:])
                    else:
                        nc.scalar.activation(o[:], ps6[s][:], Act.Copy)
                    nc.sync.dma_start(num_d[m][:, s * 512:(s + 1) * 512], o[:])

    nc.compile()
    return nc


def prep_inputs(input, sigma, train_data, n_cores=N_CORES):
    """Host-side shard + pre-tile. Returns list of per-core in_maps."""
    x = np.asarray(input, dtype=np.float32).reshape(B, D)
    sig = np.asarray(sigma, dtype=np.float64)
    y = np.asarray(train_data, dtype=np.float32).reshape(N_TOTAL, D)

    is2 = (1.0 / sig ** 2).astype(np.float32)                  # [256]
    xt16 = (x * is2[:, None]).astype(np.float16)               # x~ = x*is2
    xst = np.ascontiguousarray(xt16.reshape(B, KT, 128).transpose(1, 2, 0))  # [KT,128,256]
    is2h = is2.astype(np.float16)
    is2l = (is2 - is2h.astype(np.float32)).astype(np.float16)
    is2r = np.stack([is2h, is2h, is2l])                        # [3, 256]

    chunks = chunk_list()
    in_maps = []
    for c in range(n_cores):
        ys = y[c * NS:(c + 1) * NS]
        ys16p = np.zeros((NSP, D), dtype=np.float16)
        ys16p[:NS] = ys.astype(np.float16)
        y2f = (-0.5 * np.einsum("ij,ij->i", ys.astype(np.float64),
                                ys.astype(np.float64))).astype(np.float32)
        y2h = np.full(NSP, PAD_Y2, dtype=np.float16)
        y2l = np.zeros(NSP, dtype=np.float16)
        y2h[:NS] = y2f.astype(np.float16)
        y2l[:NS] = (y2f - y2h[:NS].astype(np.float32)).astype(np.float16)
        y2t = np.stack([y2h, y2l, y2h])                        # [3, NSP]

        ynat = np.zeros((NSP, DAUG), dtype=np.float16)
        ynat[:, :D] = ys16p

        pp, jj = np.meshgrid(np.arange(128), np.arange(CMAX), indexing="ij")
        slotr = (((pp % 8) * 8 + jj) * 16 + pp // 8).astype(np.int32)
        im = {"xst": xst, "is2r": is2r, "y2t": y2t, "ynat": ynat, "slotr": slotr}
        for ci, (coff, csz) in enumerate(chunks):
            yt = ys16p[coff:coff + csz].T.reshape(2, KH, 128, csz)
            im[f"y_c{ci}"] = np.ascontiguousarray(yt.transpose(0, 2, 1, 3))
        in_maps.append(im)
    return in_maps


def combine(results):
    """Flash-style combine of per-core (num, den, mx) partials -> full output."""
    num = np.stack([r["num"].reshape(B, D) for r in results]).astype(np.float64)
    den = np.stack([r["den"].reshape(B) for r in results]).astype(np.float64)
    mx = np.stack([r["mx"].reshape(B) for r in results]).astype(np.float64)
    M = mx.max(axis=0)
    r = np.exp(mx - M[None, :])
    num_tot = (num * r[:, :, None]).sum(axis=0)
    den_tot = (den * r).sum(axis=0)
    out = (num_tot / den_tot[:, None]).astype(np.float32)
    return out.reshape(B, C, H, W_IMG)


_NC_CACHE = {}


def get_nc():
    if "nc" not in _NC_CACHE:
        _NC_CACHE["nc"] = build_nc()
    return _NC_CACHE["nc"]


def kernel(input, sigma, train_data):
    nc = get_nc()
    in_maps = prep_inputs(input, sigma, train_data)
    res = run_bass_kernel_spmd(nc, in_maps, core_ids=list(range(N_CORES)))
    return combine(res.results)
